# revision 9
# baseline (speedup 1.0000x reference)
"""Distributed Trainium2 kernel for nn_AltBlock (dense transformer block).

Sharding: sequence-parallel across 8 cores. Core c owns 256 query tokens of
batch c//4 (quarter (c%4) of the sequence). qkv/proj/mlp run per-core on the
local tokens with full (replicated) weights; attention needs all keys/values
of the batch, obtained with ONE AllGather of (normalized K^T, V) inside each
4-core batch group. Attention is computed in the transposed layout
S^T = [k_tokens(part), q_tokens(free)] so the padding mask and softmax
normalizer are partition-friendly, alibi is added via an identity-matmul into
PSUM, and P^T feeds the V-matmul directly (no P transpose).

All matmuls use float32r operands (f32 bits, full PE rate at free-dim>=256).
"""

import math
import numpy as np
from contextlib import ExitStack

B, N, C, H = 2, 1024, 1024, 16
D = C // H          # 64
HID = 4 * C         # 4096
NCORES = 8
GROUP = 4           # cores per batch
TLOC = N // GROUP   # 256 local (query) tokens per core
EPS = 1e-5

_CACHE = {}


def _build_nc():
    import concourse.bass as bass
    import concourse.tile as tile
    from concourse import bacc, mybir

    f32 = mybir.dt.float32
    f32r = mybir.dt.float32r
    AF = mybir.ActivationFunctionType
    OP = mybir.AluOpType

    nc = bacc.Bacc(None, target_bir_lowering=False)

    x_in = nc.dram_tensor("x_loc", [TLOC, C], f32, kind="ExternalInput")
    alibi_in = nc.dram_tensor("alibi_t", [H, N, TLOC], f32r, kind="ExternalInput")
    wqkv_in = nc.dram_tensor("wqkv", [C, 3 * C], f32r, kind="ExternalInput")
    wproj_in = nc.dram_tensor("wproj", [C, C], f32r, kind="ExternalInput")
    w1_in = nc.dram_tensor("w1", [C, HID], f32r, kind="ExternalInput")
    w2_in = nc.dram_tensor("w2", [HID, C], f32r, kind="ExternalInput")
    bqkv_in = nc.dram_tensor("bqkv_t", [128, 24], f32, kind="ExternalInput")
    bproj_in = nc.dram_tensor("bproj_t", [128, 8], f32, kind="ExternalInput")
    b1_in = nc.dram_tensor("b1_t", [128, 32], f32, kind="ExternalInput")
    b2_in = nc.dram_tensor("b2_t", [128, 8], f32, kind="ExternalInput")
    ln1g_in = nc.dram_tensor("ln1_g", [1, C], f32r, kind="ExternalInput")
    ln1b_in = nc.dram_tensor("ln1_b", [1, C], f32r, kind="ExternalInput")
    ln2g_in = nc.dram_tensor("ln2_g", [1, C], f32r, kind="ExternalInput")
    ln2b_in = nc.dram_tensor("ln2_b", [1, C], f32r, kind="ExternalInput")
    scales_in = nc.dram_tensor("scales", [1, H], f32, kind="ExternalInput")
    nbound_in = nc.dram_tensor("nbound", [128, H], f32, kind="ExternalInput")
    consts_in = nc.dram_tensor("consts", [128, 192], f32r, kind="ExternalInput")
    out_ext = nc.dram_tensor("out", [TLOC, C], f32, kind="ExternalOutput")

    def bcast_ap(handle):
        ap = handle[:]
        return bass.AP(tensor=ap.tensor, offset=ap.offset, ap=[[0, 128], [1, C]])

    with ExitStack() as stack:
        stack.enter_context(nc.allow_low_precision(reason="f32r views of f32"))
        tc = stack.enter_context(tile.TileContext(nc))
        pconst = stack.enter_context(tc.tile_pool(name="pconst", bufs=1))
        pdram = stack.enter_context(tc.tile_pool(name="pdram", bufs=1, space="DRAM"))
        psA = stack.enter_context(tc.tile_pool(name="psA", bufs=4, space="PSUM"))
        psS = stack.enter_context(tc.tile_pool(name="psS", bufs=2, space="PSUM"))
        psC = stack.enter_context(tc.tile_pool(name="psC", bufs=1, space="PSUM"))

        # ---- constants / persistents ----
        consts_sb = pconst.tile([128, 192], f32r, name="consts_sb")
        nc.sync.dma_start(consts_sb, consts_in[:])
        ident = consts_sb[:, 0:128]
        ones_64_1 = consts_sb[0:64, 128:129]
        ones_1_64 = consts_sb[0:1, 128:192]
        ones_128_64 = consts_sb[:, 128:192]

        g1_sb = pconst.tile([128, C], f32r, name="g1_sb")
        nc.sync.dma_start(g1_sb, bcast_ap(ln1g_in))
        b1ln_sb = pconst.tile([128, C], f32r, name="b1ln_sb")
        nc.sync.dma_start(b1ln_sb, bcast_ap(ln1b_in))
        g2_sb = pconst.tile([128, C], f32r, name="g2_sb")
        nc.sync.dma_start(g2_sb, bcast_ap(ln2g_in))
        b2ln_sb = pconst.tile([128, C], f32r, name="b2ln_sb")
        nc.sync.dma_start(b2ln_sb, bcast_ap(ln2b_in))

        bqkv_sb = pconst.tile([128, 24], f32, name="bqkv_sb")
        nc.sync.dma_start(bqkv_sb, bqkv_in[:])
        bproj_sb = pconst.tile([128, 8], f32, name="bproj_sb")
        nc.sync.dma_start(bproj_sb, bproj_in[:])
        b1_sb = pconst.tile([128, 32], f32, name="b1_sb")
        nc.sync.dma_start(b1_sb, b1_in[:])
        b2_sb = pconst.tile([128, 8], f32, name="b2_sb")
        nc.sync.dma_start(b2_sb, b2_in[:])
        scales_sb = pconst.tile([1, H], f32, name="scales_sb")
        nc.sync.dma_start(scales_sb, scales_in[:])
        nbound_sb = pconst.tile([128, H], f32, name="nbound_sb")
        nc.sync.dma_start(nbound_sb, nbound_in[:])
        eps_sb = pconst.tile([128, 1], f32, name="eps_sb")
        nc.vector.memset(eps_sb, EPS)

        x_sb = pconst.tile([128, 2, C], f32, name="x_sb")
        for tt in range(2):
            nc.sync.dma_start(x_sb[:, tt, :], x_in[tt * 128:(tt + 1) * 128, :])
        x1_sb = pconst.tile([128, 2, C], f32, name="x1_sb")
        qnT = pconst.tile([128, 8, TLOC], f32r, name="qnT")
        OT_sb = pconst.tile([128, 8, TLOC], f32r, name="OT_sb")
        projT = pconst.tile([128, 8, TLOC], f32r, name="projT")
        out_sb = pconst.tile([128, 2, C], f32, name="out_sb")

        def layernorm(pool, x_slice, g_t, b_t, out_t):
            stats = pool.tile([128, 2, 6], f32, name="lnstats", tag="lnstats")
            for sg in range(2):
                nc.vector.bn_stats(out=stats[:, sg, :],
                                   in_=x_slice[:, sg * 512:(sg + 1) * 512])
            mv = pool.tile([128, 2], f32, name="lnmv", tag="lnmv")
            nc.vector.bn_aggr(out=mv, in_=stats)
            rstd = pool.tile([128, 1], f32, name="lnrstd", tag="lnrstd")
            nc.scalar.activation(out=rstd, in_=mv[:, 1:2], func=AF.Sqrt,
                                 bias=eps_sb[:, 0:1])
            nc.vector.reciprocal(out=rstd, in_=rstd)
            tmp = pool.tile([128, C], f32, name="lntmp", tag="lntmp")
            nc.vector.tensor_scalar(out=tmp, in0=x_slice, scalar1=mv[:, 0:1],
                                    scalar2=rstd, op0=OP.subtract, op1=OP.mult)
            nc.vector.tensor_mul(tmp, tmp, g_t)
            nc.vector.tensor_add(out_t, tmp, b_t)

        def norm_heads(pool, src_col0, dst, with_scale):
            # src: qkv_sb column blocks [128, 8, TLOC] starting at block src_col0
            # dst: [128, 8, TLOC] f32r normalized (q: * scale_h)
            for h in range(H):
                rows = slice(64 * (h % 2), 64 * (h % 2) + 64)
                th = qkv_sb[rows, src_col0 + h // 2, :]
                q2 = pool.tile([64, TLOC], f32r, name="q2", tag="q2")
                nc.vector.tensor_mul(q2, th, th)
                ssq = psC.tile([1, TLOC], f32, name="ssq", tag="small1")
                nc.tensor.matmul(ssq, lhsT=ones_64_1, rhs=q2)
                rn = pool.tile([1, TLOC], f32, name="rn", tag="rn")
                nc.scalar.activation(out=rn, in_=ssq, func=AF.Sqrt)
                nc.vector.tensor_scalar(out=rn, in0=rn, scalar1=1e-12,
                                        scalar2=None, op0=OP.max)
                nc.vector.reciprocal(rn, rn)
                if with_scale:
                    nc.vector.tensor_scalar(out=rn, in0=rn,
                                            scalar1=scales_sb[0:1, h:h + 1],
                                            scalar2=None, op0=OP.mult)
                rnr = pool.tile([1, TLOC], f32r, name="rnr", tag="rnr")
                nc.vector.tensor_copy(rnr, rn)
                bc = psC.tile([64, TLOC], f32, name="bc", tag="small2")
                nc.tensor.matmul(bc, lhsT=ones_1_64, rhs=rnr)
                nc.vector.tensor_mul(dst[rows, h // 2, :], bc, th)

        # ================= Phase A: LN1, qkv, normalize, pack =================
        bounce_in = pdram.tile([128, 4096], f32r, name="bounce_in")
        with tc.tile_pool(name="pA", bufs=1) as pA, \
             tc.tile_pool(name="wstA", bufs=3) as wstA, \
             tc.tile_pool(name="ptmpA", bufs=4) as ptmpA:
            h_sb = pA.tile([128, 2, C], f32r, name="h_sb")
            for tt in range(2):
                layernorm(ptmpA, x_sb[:, tt, :], g1_sb, b1ln_sb, h_sb[:, tt, :])
            hT = pA.tile([128, 8, TLOC], f32r, name="hT")
            for tt in range(2):
                for cc in range(8):
                    tp = psA.tile([128, 128], f32r, name="tp", tag="mm")
                    nc.tensor.transpose(tp, h_sb[:, tt, cc * 128:(cc + 1) * 128], ident)
                    nc.vector.tensor_copy(hT[:, cc, tt * 128:(tt + 1) * 128], tp)

            qkv_sb = pA.tile([128, 24, TLOC], f32r, name="qkv_sb")
            for sup in range(6):
                pss = [psA.tile([128, TLOC], f32, name=f"qps{blk}", tag="mm")
                       for blk in range(4)]
                for cc in range(8):
                    wq = wstA.tile([128, 512], f32r, name="wq", tag="w")
                    nc.sync.dma_start(wq, wqkv_in[cc * 128:(cc + 1) * 128,
                                                  sup * 512:(sup + 1) * 512])
                    for blk in range(4):
                        nc.tensor.matmul(pss[blk],
                                         lhsT=wq[:, blk * 128:(blk + 1) * 128],
                                         rhs=hT[:, cc, :],
                                         start=(cc == 0), stop=(cc == 7))
                for blk in range(4):
                    cb = sup * 4 + blk
                    nc.vector.tensor_scalar(out=qkv_sb[:, cb, :], in0=pss[blk],
                                            scalar1=bqkv_sb[:, cb:cb + 1],
                                            scalar2=None, op0=OP.add)

            # normalize k heads -> knT_loc; transpose v back to token-major
            knT_loc = pA.tile([128, 8, TLOC], f32r, name="knT_loc")
            norm_heads(ptmpA, 8, knT_loc, with_scale=False)
            v_loc = pA.tile([128, 2, C], f32r, name="v_loc")
            for cb in range(8):
                for tt in range(2):
                    tp2 = psA.tile([128, 128], f32r, name="tp2", tag="mm")
                    nc.tensor.transpose(tp2, qkv_sb[:, 16 + cb, tt * 128:(tt + 1) * 128], ident)
                    nc.vector.tensor_copy(v_loc[:, tt, cb * 128:(cb + 1) * 128], tp2)
            nc.sync.dma_start(bounce_in[:, 0:2048],
                              knT_loc.rearrange("p a b -> p (a b)"))
            nc.sync.dma_start(bounce_in[:, 2048:4096],
                              v_loc.rearrange("p a b -> p (a b)"))
            # normalize q heads (overlaps the collective)
            norm_heads(ptmpA, 0, qnT, with_scale=True)

        # ================= Phase B: AllGather, attention, proj ================
        with tc.tile_pool(name="pB", bufs=1) as pB, \
             tc.tile_pool(name="alst", bufs=6) as alst, \
             tc.tile_pool(name="ppt", bufs=2) as ppt, \
             tc.tile_pool(name="wstB", bufs=3) as wstB, \
             tc.tile_pool(name="ptmpB", bufs=4) as ptmpB:
            bounce_out = pdram.tile([512, 4096], f32r, name="bounce_out")
            nc.gpsimd.collective_compute(
                "AllGather", OP.bypass,
                ins=[bounce_in.opt()],
                outs=[bounce_out.opt()],
                replica_groups=[[0, 1, 2, 3], [4, 5, 6, 7]],
            )
            knv = []
            for r in range(4):
                t = pB.tile([128, 4096], f32r, name=f"knv{r}", tag=f"knv{r}")
                nc.sync.dma_start(t, bounce_out[r * 128:(r + 1) * 128, :])
                knv.append(t)

            for h in range(H):
                rows = slice(64 * (h % 2), 64 * (h % 2) + 64)
                P_T = ppt.tile([128, 8, TLOC], f32r, name="P_T", tag="pt")
                OT_ps = psC.tile([64, TLOC], f32, name="OT_ps", tag="small1")
                sums_ps = psC.tile([64, TLOC], f32, name="sums_ps", tag="small2")
                for kc in range(8):
                    r, tt = kc // 2, kc % 2
                    al = alst.tile([128, TLOC], f32r, name="al", tag="al")
                    nc.sync.dma_start(al, alibi_in[h, kc * 128:(kc + 1) * 128, :])
                    S = psS.tile([128, TLOC], f32, name="S", tag="s")
                    knT_sl = knv[r][rows, (h // 2) * 256 + tt * 128:
                                    (h // 2) * 256 + tt * 128 + 128]
                    nc.tensor.matmul(S, lhsT=knT_sl, rhs=qnT[rows, h // 2, :],
                                     start=True, stop=False)
                    nc.tensor.matmul(S, lhsT=ident, rhs=al,
                                     start=False, stop=True)
                    nc.scalar.activation(out=P_T[:, kc, :], in_=S, func=AF.Exp,
                                         bias=nbound_sb[:, h:h + 1], scale=1.0)
                    v_sl = knv[r][:, 2048 + tt * 1024 + h * 64:
                                  2048 + tt * 1024 + h * 64 + 64]
                    nc.tensor.matmul(OT_ps, lhsT=v_sl, rhs=P_T[:, kc, :],
                                     start=(kc == 0), stop=(kc == 7))
                    nc.tensor.matmul(sums_ps, lhsT=ones_128_64, rhs=P_T[:, kc, :],
                                     start=(kc == 0), stop=(kc == 7))
                rs = ptmpB.tile([64, TLOC], f32, name="rs", tag="rs")
                nc.vector.reciprocal(rs, sums_ps)
                nc.vector.tensor_mul(OT_sb[rows, h // 2, :], OT_ps, rs)

            # proj: projT[cout,t] = Wproj^T @ O^T
            for half in range(2):
                pss = [psA.tile([128, TLOC], f32, name=f"pps{blk}", tag="mm")
                       for blk in range(4)]
                for cc in range(8):
                    wp = wstB.tile([128, 512], f32r, name="wp", tag="w")
                    nc.sync.dma_start(wp, wproj_in[cc * 128:(cc + 1) * 128,
                                                   half * 512:(half + 1) * 512])
                    for blk in range(4):
                        nc.tensor.matmul(pss[blk],
                                         lhsT=wp[:, blk * 128:(blk + 1) * 128],
                                         rhs=OT_sb[:, cc, :],
                                         start=(cc == 0), stop=(cc == 7))
                for blk in range(4):
                    cb = half * 4 + blk
                    nc.vector.tensor_scalar(out=projT[:, cb, :], in0=pss[blk],
                                            scalar1=bproj_sb[:, cb:cb + 1],
                                            scalar2=None, op0=OP.add)
            # x1 = x + attn_out (transpose projT back to token-major)
            for tt in range(2):
                for cb in range(8):
                    tp3 = psA.tile([128, 128], f32r, name="tp3", tag="mm")
                    nc.tensor.transpose(tp3, projT[:, cb, tt * 128:(tt + 1) * 128], ident)
                    nc.vector.tensor_add(x1_sb[:, tt, cb * 128:(cb + 1) * 128],
                                         tp3, x_sb[:, tt, cb * 128:(cb + 1) * 128])

        # ================= Phase C: LN2 + MLP =================
        with tc.tile_pool(name="pC", bufs=1) as pC, \
             tc.tile_pool(name="wstC", bufs=3) as wstC, \
             tc.tile_pool(name="ptmpC", bufs=4) as ptmpC:
            y_sb = pC.tile([128, 2, C], f32r, name="y_sb")
            for tt in range(2):
                layernorm(ptmpC, x1_sb[:, tt, :], g2_sb, b2ln_sb, y_sb[:, tt, :])
            yT = pC.tile([128, 8, TLOC], f32r, name="yT")
            for tt in range(2):
                for cc in range(8):
                    tp4 = psA.tile([128, 128], f32r, name="tp4", tag="mm")
                    nc.tensor.transpose(tp4, y_sb[:, tt, cc * 128:(cc + 1) * 128], ident)
                    nc.vector.tensor_copy(yT[:, cc, tt * 128:(tt + 1) * 128], tp4)

            h1 = pC.tile([128, 32, TLOC], f32r, name="h1")
            for sup in range(8):
                pss = [psA.tile([128, TLOC], f32, name=f"m1ps{blk}", tag="mm")
                       for blk in range(4)]
                for cc in range(8):
                    w1t = wstC.tile([128, 512], f32r, name="w1t", tag="w")
                    nc.sync.dma_start(w1t, w1_in[cc * 128:(cc + 1) * 128,
                                                 sup * 512:(sup + 1) * 512])
                    for blk in range(4):
                        nc.tensor.matmul(pss[blk],
                                         lhsT=w1t[:, blk * 128:(blk + 1) * 128],
                                         rhs=yT[:, cc, :],
                                         start=(cc == 0), stop=(cc == 7))
                for blk in range(4):
                    hb = sup * 4 + blk
                    nc.scalar.activation(out=h1[:, hb, :], in_=pss[blk],
                                         func=AF.Gelu,
                                         bias=b1_sb[:, hb:hb + 1], scale=1.0)

            y2T = pC.tile([128, 8, TLOC], f32r, name="y2T")
            for half in range(2):
                pss = [psA.tile([128, TLOC], f32, name=f"m2ps{blk}", tag="mm")
                       for blk in range(4)]
                for hc in range(32):
                    w2t = wstC.tile([128, 512], f32r, name="w2t", tag="w")
                    nc.sync.dma_start(w2t, w2_in[hc * 128:(hc + 1) * 128,
                                                 half * 512:(half + 1) * 512])
                    for blk in range(4):
                        nc.tensor.matmul(pss[blk],
                                         lhsT=w2t[:, blk * 128:(blk + 1) * 128],
                                         rhs=h1[:, hc, :],
                                         start=(hc == 0), stop=(hc == 31))
                for blk in range(4):
                    cb = half * 4 + blk
                    nc.vector.tensor_scalar(out=y2T[:, cb, :], in0=pss[blk],
                                            scalar1=b2_sb[:, cb:cb + 1],
                                            scalar2=None, op0=OP.add)
            for tt in range(2):
                for cb in range(8):
                    tp5 = psA.tile([128, 128], f32r, name="tp5", tag="mm")
                    nc.tensor.transpose(tp5, y2T[:, cb, tt * 128:(tt + 1) * 128], ident)
                    nc.vector.tensor_add(out_sb[:, tt, cb * 128:(cb + 1) * 128],
                                         tp5, x1_sb[:, tt, cb * 128:(cb + 1) * 128])
            for tt in range(2):
                nc.sync.dma_start(out_ext[tt * 128:(tt + 1) * 128, :],
                                  out_sb[:, tt, :])

    nc.finalize()
    return nc


def _get_nc():
    if "nc" not in _CACHE:
        _CACHE["nc"] = _build_nc()
    return _CACHE["nc"]


def _make_in_maps(inputs):
    x = np.asarray(inputs["x"], np.float32)
    mask = np.asarray(inputs["padding_mask"]).astype(bool)
    alibi = np.asarray(inputs["alibi_bias"], np.float32)
    wqkv = np.ascontiguousarray(np.asarray(inputs["Wqkv"], np.float32))
    bqkv = np.asarray(inputs["bqkv"], np.float32)
    wproj = np.ascontiguousarray(np.asarray(inputs["Wproj"], np.float32))
    bproj = np.asarray(inputs["bproj"], np.float32)
    w1 = np.ascontiguousarray(np.asarray(inputs["W1"], np.float32))
    b1 = np.asarray(inputs["b1"], np.float32)
    w2 = np.ascontiguousarray(np.asarray(inputs["W2"], np.float32))
    b2 = np.asarray(inputs["b2"], np.float32)
    ls = np.asarray(inputs["logit_scale"], np.float32).reshape(H)
    scale = np.exp(np.minimum(ls, math.log(100.0))).astype(np.float32)
    amax = float(alibi.max())
    bound = scale + amax + 1.0
    nbound = np.tile((-bound).astype(np.float32)[None, :], (128, 1))
    nbound = np.ascontiguousarray(nbound)

    common = {
        "wqkv": wqkv,
        "bqkv_t": np.ascontiguousarray(bqkv.reshape(24, 128).T),
        "wproj": wproj,
        "bproj_t": np.ascontiguousarray(bproj.reshape(8, 128).T),
        "w1": w1,
        "b1_t": np.ascontiguousarray(b1.reshape(32, 128).T),
        "w2": w2,
        "b2_t": np.ascontiguousarray(b2.reshape(8, 128).T),
        "ln1_g": np.asarray(inputs["ln1_g"], np.float32).reshape(1, C),
        "ln1_b": np.asarray(inputs["ln1_b"], np.float32).reshape(1, C),
        "ln2_g": np.asarray(inputs["ln2_g"], np.float32).reshape(1, C),
        "ln2_b": np.asarray(inputs["ln2_b"], np.float32).reshape(1, C),
        "scales": scale.reshape(1, H),
        "nbound": nbound,
        "consts": np.ascontiguousarray(np.concatenate(
            [np.eye(128, dtype=np.float32),
             np.ones((128, 64), dtype=np.float32)], axis=1)),
    }
    in_maps = []
    for c in range(NCORES):
        b, qi = divmod(c, GROUP)
        q0 = qi * TLOC
        alT = np.ascontiguousarray(
            alibi[b, :, q0:q0 + TLOC, :].transpose(0, 2, 1))  # [H, N(k), TLOC(q)]
        alT = alT + np.where(mask[b], np.float32(-1e9),
                             np.float32(0.0)).astype(np.float32)[None, :, None]
        m = dict(common)
        m["x_loc"] = np.ascontiguousarray(x[b, q0:q0 + TLOC, :])
        m["alibi_t"] = np.ascontiguousarray(alT)
        in_maps.append(m)
    return in_maps


def _run(inputs, trace=False):
    from concourse import bass_utils
    nc = _get_nc()
    in_maps = _make_in_maps(inputs)
    res = bass_utils.run_bass_kernel_spmd(
        nc, in_maps, core_ids=list(range(NCORES)), trace=trace)
    outs = [np.asarray(res.results[c]["out"]) for c in range(NCORES)]
    y = np.stack(outs).reshape(B, GROUP * TLOC, C)
    return y.astype(np.float32), res


def kernel(**inputs):
    y, _ = _run(inputs, trace=False)
    return y


# revision 10
# speedup vs baseline: 1.0368x; 1.0368x over previous
"""Distributed Trainium2 kernel for nn_AltBlock (dense transformer block).

Sharding: sequence-parallel across 8 cores. Core c owns 256 query tokens of
batch c//4 (quarter (c%4) of the sequence). qkv/proj/mlp run per-core on the
local tokens with full (replicated) weights; attention needs all keys/values
of the batch, obtained with TWO small bf16 AllGathers (kn^T, then V) inside
each 4-core batch group, pipelined against S-matrix compute. Attention is
computed in the transposed layout S^T = [k_tokens(part), q_tokens(free)] so
the padding mask and softmax normalizer are partition-friendly, alibi is
added via an identity-matmul into PSUM, and P^T feeds the V-matmul directly.

Matmuls use float32r (f32 bits, full PE rate at free-dim>=256) on the
weight path and bfloat16 on the attention path. Weights are host-pre-tiled
into contiguous [128,512] DMA tiles.
"""

import math
import numpy as np
from contextlib import ExitStack

B, N, C, H = 2, 1024, 1024, 16
D = C // H          # 64
HID = 4 * C         # 4096
NCORES = 8
GROUP = 4           # cores per batch
TLOC = N // GROUP   # 256 local (query) tokens per core
EPS = 1e-5

_CACHE = {}


def _build_nc():
    import concourse.bass as bass
    import concourse.tile as tile
    from concourse import bacc, mybir

    f32 = mybir.dt.float32
    f32r = mybir.dt.float32r
    bf16 = mybir.dt.bfloat16
    AF = mybir.ActivationFunctionType
    OP = mybir.AluOpType

    nc = bacc.Bacc(None, target_bir_lowering=False)

    x_in = nc.dram_tensor("x_loc", [TLOC, C], f32, kind="ExternalInput")
    alibi_in = nc.dram_tensor("alibi_t", [H, 8, 128, TLOC], bf16, kind="ExternalInput")
    wqkv_in = nc.dram_tensor("wqkv_t", [8, 6, 128, 512], f32r, kind="ExternalInput")
    wproj_in = nc.dram_tensor("wproj_t", [8, 2, 128, 512], f32r, kind="ExternalInput")
    w1_in = nc.dram_tensor("w1_t", [8, 8, 128, 512], f32r, kind="ExternalInput")
    w2_in = nc.dram_tensor("w2_t", [32, 2, 128, 512], f32r, kind="ExternalInput")
    bqkv_in = nc.dram_tensor("bqkv_t", [128, 24], f32, kind="ExternalInput")
    bproj_in = nc.dram_tensor("bproj_t", [128, 8], f32, kind="ExternalInput")
    b1_in = nc.dram_tensor("b1_t", [128, 32], f32, kind="ExternalInput")
    b2_in = nc.dram_tensor("b2_t", [128, 8], f32, kind="ExternalInput")
    ln1g_in = nc.dram_tensor("ln1_g", [1, C], f32r, kind="ExternalInput")
    ln1b_in = nc.dram_tensor("ln1_b", [1, C], f32r, kind="ExternalInput")
    ln2g_in = nc.dram_tensor("ln2_g", [1, C], f32r, kind="ExternalInput")
    ln2b_in = nc.dram_tensor("ln2_b", [1, C], f32r, kind="ExternalInput")
    scales_in = nc.dram_tensor("scales", [1, H], f32, kind="ExternalInput")
    nbound_in = nc.dram_tensor("nbound", [128, H], f32, kind="ExternalInput")
    consts_in = nc.dram_tensor("consts", [128, 192], f32r, kind="ExternalInput")
    constsb_in = nc.dram_tensor("consts_bf", [128, 192], bf16, kind="ExternalInput")
    out_ext = nc.dram_tensor("out", [TLOC, C], f32, kind="ExternalOutput")

    def bcast_ap(handle):
        ap = handle[:]
        return bass.AP(tensor=ap.tensor, offset=ap.offset, ap=[[0, 128], [1, C]])

    with ExitStack() as stack:
        stack.enter_context(nc.allow_low_precision(reason="f32r views of f32"))
        tc = stack.enter_context(tile.TileContext(nc))
        pconst = stack.enter_context(tc.tile_pool(name="pconst", bufs=1))
        pdram = stack.enter_context(tc.tile_pool(name="pdram", bufs=1, space="DRAM"))
        psA = stack.enter_context(tc.tile_pool(name="psA", bufs=4, space="PSUM"))
        psS = stack.enter_context(tc.tile_pool(name="psS", bufs=2, space="PSUM"))

        # ---- constants / persistents ----
        consts_sb = pconst.tile([128, 192], f32r, name="consts_sb")
        nc.sync.dma_start(consts_sb, consts_in[:])
        ident = consts_sb[:, 0:128]
        ones_64_1 = consts_sb[0:64, 128:129]
        ones_1_64 = consts_sb[0:1, 128:192]
        constsb_sb = pconst.tile([128, 192], bf16, name="constsb_sb")
        nc.sync.dma_start(constsb_sb, constsb_in[:])
        ident_bf = constsb_sb[:, 0:128]
        ones_128_64_bf = constsb_sb[:, 128:192]

        g1_sb = pconst.tile([128, C], f32r, name="g1_sb")
        nc.sync.dma_start(g1_sb, bcast_ap(ln1g_in))
        b1ln_sb = pconst.tile([128, C], f32r, name="b1ln_sb")
        nc.sync.dma_start(b1ln_sb, bcast_ap(ln1b_in))
        g2_sb = pconst.tile([128, C], f32r, name="g2_sb")
        nc.sync.dma_start(g2_sb, bcast_ap(ln2g_in))
        b2ln_sb = pconst.tile([128, C], f32r, name="b2ln_sb")
        nc.sync.dma_start(b2ln_sb, bcast_ap(ln2b_in))

        bqkv_sb = pconst.tile([128, 24], f32, name="bqkv_sb")
        nc.sync.dma_start(bqkv_sb, bqkv_in[:])
        bproj_sb = pconst.tile([128, 8], f32, name="bproj_sb")
        nc.sync.dma_start(bproj_sb, bproj_in[:])
        b1_sb = pconst.tile([128, 32], f32, name="b1_sb")
        nc.sync.dma_start(b1_sb, b1_in[:])
        b2_sb = pconst.tile([128, 8], f32, name="b2_sb")
        nc.sync.dma_start(b2_sb, b2_in[:])
        scales_sb = pconst.tile([1, H], f32, name="scales_sb")
        nc.sync.dma_start(scales_sb, scales_in[:])
        nbound_sb = pconst.tile([128, H], f32, name="nbound_sb")
        nc.sync.dma_start(nbound_sb, nbound_in[:])
        eps_sb = pconst.tile([128, 1], f32, name="eps_sb")
        nc.vector.memset(eps_sb, EPS)

        x_sb = pconst.tile([128, 2, C], f32, name="x_sb")
        for tt in range(2):
            nc.sync.dma_start(x_sb[:, tt, :], x_in[tt * 128:(tt + 1) * 128, :])
        x1_sb = pconst.tile([128, 2, C], f32, name="x1_sb")
        qnT = pconst.tile([128, 8, TLOC], bf16, name="qnT")
        OT_sb = pconst.tile([128, 8, TLOC], f32r, name="OT_sb")
        projT = pconst.tile([128, 8, TLOC], f32r, name="projT")
        out_sb = pconst.tile([128, 2, C], f32, name="out_sb")

        def layernorm(pool, x_slice, g_t, b_t, out_t):
            stats = pool.tile([128, 2, 6], f32, name="lnstats", tag="lnstats")
            for sg in range(2):
                nc.vector.bn_stats(out=stats[:, sg, :],
                                   in_=x_slice[:, sg * 512:(sg + 1) * 512])
            mv = pool.tile([128, 2], f32, name="lnmv", tag="lnmv")
            nc.vector.bn_aggr(out=mv, in_=stats)
            rstd = pool.tile([128, 1], f32, name="lnrstd", tag="lnrstd")
            nc.scalar.activation(out=rstd, in_=mv[:, 1:2], func=AF.Sqrt,
                                 bias=eps_sb[:, 0:1])
            nc.vector.reciprocal(out=rstd, in_=rstd)
            tmp = pool.tile([128, C], f32, name="lntmp", tag="lntmp")
            nc.vector.tensor_scalar(out=tmp, in0=x_slice, scalar1=mv[:, 0:1],
                                    scalar2=rstd, op0=OP.subtract, op1=OP.mult)
            nc.vector.tensor_mul(tmp, tmp, g_t)
            nc.vector.tensor_add(out_t, tmp, b_t)

        def qkv_super(sup, qkv_sb, hT):
            # one 512-col super-block of the qkv matmul, accumulated over C
            pss = [psA.tile([128, TLOC], f32, name=f"qps{blk}", tag="mm")
                   for blk in range(4)]
            for cc in range(8):
                wq = wstA.tile([128, 512], f32r, name="wq", tag="w")
                nc.sync.dma_start(wq, wqkv_in[cc, sup])
                for blk in range(4):
                    nc.tensor.matmul(pss[blk],
                                     lhsT=wq[:, blk * 128:(blk + 1) * 128],
                                     rhs=hT[:, cc, :],
                                     start=(cc == 0), stop=(cc == 7))
            for blk in range(4):
                cb = sup * 4 + blk
                nc.scalar.activation(out=qkv_sb[:, cb, :], in_=pss[blk],
                                     func=AF.Identity,
                                     bias=bqkv_sb[:, cb:cb + 1], scale=1.0)

        def norm_heads(pool, qkv_sb, src_col0, dst, with_scale):
            for h in range(H):
                rows = slice(64 * (h % 2), 64 * (h % 2) + 64)
                th = qkv_sb[rows, src_col0 + h // 2, :]
                q2 = pool.tile([64, TLOC], f32r, name="q2", tag="q2")
                nc.vector.tensor_mul(q2, th, th)
                ssq = psS.tile([1, TLOC], f32, name="ssq", tag="acc")
                nc.tensor.matmul(ssq, lhsT=ones_64_1, rhs=q2)
                rn = pool.tile([1, TLOC], f32, name="rn", tag="rn")
                nc.scalar.activation(out=rn, in_=ssq, func=AF.Sqrt)
                nc.vector.tensor_scalar(out=rn, in0=rn, scalar1=1e-12,
                                        scalar2=None, op0=OP.max)
                nc.vector.reciprocal(rn, rn)
                if with_scale:
                    nc.vector.tensor_scalar(out=rn, in0=rn,
                                            scalar1=scales_sb[0:1, h:h + 1],
                                            scalar2=None, op0=OP.mult)
                rnr = pool.tile([1, TLOC], f32r, name="rnr", tag="rnr")
                nc.vector.tensor_copy(rnr, rn)
                bc = psS.tile([64, TLOC], f32, name="bc", tag="acc")
                nc.tensor.matmul(bc, lhsT=ones_1_64, rhs=rnr)
                nc.vector.tensor_mul(dst[rows, h // 2, :], bc, th)

        # ============== Phase A: LN1, qkv (kv first), pack, AGs ==============
        bounce_kn = pdram.tile([128, 2048], bf16, name="bounce_kn")
        bounce_v = pdram.tile([128, 2048], bf16, name="bounce_v")
        with tc.tile_pool(name="pA", bufs=1) as pA, \
             tc.tile_pool(name="wstA", bufs=3) as wstA, \
             tc.tile_pool(name="ptmpA", bufs=4) as ptmpA:
            h_sb = pA.tile([128, 2, C], f32r, name="h_sb")
            for tt in range(2):
                layernorm(ptmpA, x_sb[:, tt, :], g1_sb, b1ln_sb, h_sb[:, tt, :])
            hT = pA.tile([128, 8, TLOC], f32r, name="hT")
            for tt in range(2):
                for cc in range(8):
                    tp = psA.tile([128, 128], f32r, name="tp", tag="mm")
                    nc.tensor.transpose(tp, h_sb[:, tt, cc * 128:(cc + 1) * 128], ident)
                    nc.scalar.activation(out=hT[:, cc, tt * 128:(tt + 1) * 128],
                                         in_=tp, func=AF.Copy)

            qkv_sb = pA.tile([128, 24, TLOC], f32r, name="qkv_sb")
            # K and V column supers first so the collectives can start early
            for sup in (2, 3, 4, 5):
                qkv_super(sup, qkv_sb, hT)
            knT_loc = pA.tile([128, 8, TLOC], bf16, name="knT_loc")
            norm_heads(ptmpA, qkv_sb, 8, knT_loc, with_scale=False)
            nc.sync.dma_start(bounce_kn, knT_loc.rearrange("p a b -> p (a b)"))
            v_loc = pA.tile([128, 2, C], bf16, name="v_loc")
            for cb in range(8):
                for tt in range(2):
                    tp2 = psA.tile([128, 128], f32r, name="tp2", tag="mm")
                    nc.tensor.transpose(tp2, qkv_sb[:, 16 + cb, tt * 128:(tt + 1) * 128],
                                        ident)
                    nc.scalar.activation(out=v_loc[:, tt, cb * 128:(cb + 1) * 128],
                                         in_=tp2, func=AF.Copy)
            nc.sync.dma_start(bounce_v, v_loc.rearrange("p a b -> p (a b)"))
            # Q column supers + q normalization (overlap the collectives)
            for sup in (0, 1):
                qkv_super(sup, qkv_sb, hT)
            norm_heads(ptmpA, qkv_sb, 0, qnT, with_scale=True)

        # ============== Phase B: attention (pipelined over AG2), proj =========
        with tc.tile_pool(name="pB", bufs=1) as pB, \
             tc.tile_pool(name="alst", bufs=4) as alst, \
             tc.tile_pool(name="wstB", bufs=3) as wstB, \
             tc.tile_pool(name="ptmpB", bufs=4) as ptmpB:
            ag_kn = pdram.tile([512, 2048], bf16, name="ag_kn")
            nc.gpsimd.collective_compute(
                "AllGather", OP.bypass,
                ins=[bounce_kn.opt()], outs=[ag_kn.opt()],
                replica_groups=[[0, 1, 2, 3], [4, 5, 6, 7]],
            )
            ag_v = pdram.tile([512, 2048], bf16, name="ag_v")
            nc.gpsimd.collective_compute(
                "AllGather", OP.bypass,
                ins=[bounce_v.opt()], outs=[ag_v.opt()],
                replica_groups=[[0, 1, 2, 3], [4, 5, 6, 7]],
            )
            kn_r, v_r = [], []
            for r in range(4):
                t = pB.tile([128, 2048], bf16, name=f"kn{r}", tag=f"kn{r}")
                nc.sync.dma_start(t, ag_kn[r * 128:(r + 1) * 128, :])
                kn_r.append(t)
            for r in range(4):
                t = pB.tile([128, 2048], bf16, name=f"v{r}", tag=f"v{r}")
                nc.sync.dma_start(t, ag_v[r * 128:(r + 1) * 128, :])
                v_r.append(t)

            P_T = pB.tile([128, H, 8, TLOC], bf16, name="P_T")
            rs_sb = pB.tile([64, H, TLOC], f32, name="rs_sb")
            # pass 1: S = kn^T q * scale + alibi (+mask), exp, row-sums
            for h in range(H):
                rows = slice(64 * (h % 2), 64 * (h % 2) + 64)
                sums_ps = psS.tile([64, TLOC], f32, name="sums_ps", tag="acc")
                for kc in range(8):
                    r, tt = kc // 2, kc % 2
                    al = alst.tile([128, TLOC], bf16, name="al", tag="al")
                    nc.sync.dma_start(al, alibi_in[h, kc])
                    S = psS.tile([128, TLOC], f32, name="S", tag="s")
                    knT_sl = kn_r[r][rows, (h // 2) * 256 + tt * 128:
                                     (h // 2) * 256 + tt * 128 + 128]
                    nc.tensor.matmul(S, lhsT=knT_sl, rhs=qnT[rows, h // 2, :],
                                     start=True, stop=False)
                    nc.tensor.matmul(S, lhsT=ident_bf, rhs=al,
                                     start=False, stop=True)
                    nc.scalar.activation(out=P_T[:, h, kc, :], in_=S, func=AF.Exp,
                                         bias=nbound_sb[:, h:h + 1], scale=1.0)
                    nc.tensor.matmul(sums_ps, lhsT=ones_128_64_bf,
                                     rhs=P_T[:, h, kc, :],
                                     start=(kc == 0), stop=(kc == 7))
                nc.vector.reciprocal(rs_sb[:, h, :], sums_ps)
            # pass 2: O^T = V^T P^T (waits on AG2), normalize by row-sums
            for h in range(H):
                rows = slice(64 * (h % 2), 64 * (h % 2) + 64)
                OT_ps = psS.tile([64, TLOC], f32, name="OT_ps", tag="acc")
                for kc in range(8):
                    r, tt = kc // 2, kc % 2
                    v_sl = v_r[r][:, tt * 1024 + h * 64: tt * 1024 + h * 64 + 64]
                    nc.tensor.matmul(OT_ps, lhsT=v_sl, rhs=P_T[:, h, kc, :],
                                     start=(kc == 0), stop=(kc == 7))
                nc.vector.tensor_mul(OT_sb[rows, h // 2, :], OT_ps, rs_sb[:, h, :])

            # proj
            for sup in range(2):
                pss = [psA.tile([128, TLOC], f32, name=f"pps{blk}", tag="mm")
                       for blk in range(4)]
                for cc in range(8):
                    wp = wstB.tile([128, 512], f32r, name="wp", tag="w")
                    nc.sync.dma_start(wp, wproj_in[cc, sup])
                    for blk in range(4):
                        nc.tensor.matmul(pss[blk],
                                         lhsT=wp[:, blk * 128:(blk + 1) * 128],
                                         rhs=OT_sb[:, cc, :],
                                         start=(cc == 0), stop=(cc == 7))
                for blk in range(4):
                    cb = sup * 4 + blk
                    nc.scalar.activation(out=projT[:, cb, :], in_=pss[blk],
                                         func=AF.Identity,
                                         bias=bproj_sb[:, cb:cb + 1], scale=1.0)
            for tt in range(2):
                for cb in range(8):
                    tp3 = psA.tile([128, 128], f32r, name="tp3", tag="mm")
                    nc.tensor.transpose(tp3, projT[:, cb, tt * 128:(tt + 1) * 128],
                                        ident)
                    nc.vector.tensor_add(x1_sb[:, tt, cb * 128:(cb + 1) * 128],
                                         tp3, x_sb[:, tt, cb * 128:(cb + 1) * 128])

        # ================= Phase C: LN2 + MLP =================
        with tc.tile_pool(name="pC", bufs=1) as pC, \
             tc.tile_pool(name="wstC", bufs=3) as wstC, \
             tc.tile_pool(name="ptmpC", bufs=4) as ptmpC:
            y_sb = pC.tile([128, 2, C], f32r, name="y_sb")
            for tt in range(2):
                layernorm(ptmpC, x1_sb[:, tt, :], g2_sb, b2ln_sb, y_sb[:, tt, :])
            yT = pC.tile([128, 8, TLOC], f32r, name="yT")
            for tt in range(2):
                for cc in range(8):
                    tp4 = psA.tile([128, 128], f32r, name="tp4", tag="mm")
                    nc.tensor.transpose(tp4, y_sb[:, tt, cc * 128:(cc + 1) * 128], ident)
                    nc.scalar.activation(out=yT[:, cc, tt * 128:(tt + 1) * 128],
                                         in_=tp4, func=AF.Copy)

            h1 = pC.tile([128, 32, TLOC], f32r, name="h1")
            for sup in range(8):
                pss = [psA.tile([128, TLOC], f32, name=f"m1ps{blk}", tag="mm")
                       for blk in range(4)]
                for cc in range(8):
                    w1t = wstC.tile([128, 512], f32r, name="w1t", tag="w")
                    nc.sync.dma_start(w1t, w1_in[cc, sup])
                    for blk in range(4):
                        nc.tensor.matmul(pss[blk],
                                         lhsT=w1t[:, blk * 128:(blk + 1) * 128],
                                         rhs=yT[:, cc, :],
                                         start=(cc == 0), stop=(cc == 7))
                for blk in range(4):
                    hb = sup * 4 + blk
                    nc.scalar.activation(out=h1[:, hb, :], in_=pss[blk],
                                         func=AF.Gelu,
                                         bias=b1_sb[:, hb:hb + 1], scale=1.0)

            y2T = pC.tile([128, 8, TLOC], f32r, name="y2T")
            for half in range(2):
                pss = [psA.tile([128, TLOC], f32, name=f"m2ps{blk}", tag="mm")
                       for blk in range(4)]
                for hc in range(32):
                    w2t = wstC.tile([128, 512], f32r, name="w2t", tag="w")
                    nc.sync.dma_start(w2t, w2_in[hc, half])
                    for blk in range(4):
                        nc.tensor.matmul(pss[blk],
                                         lhsT=w2t[:, blk * 128:(blk + 1) * 128],
                                         rhs=h1[:, hc, :],
                                         start=(hc == 0), stop=(hc == 31))
                for blk in range(4):
                    cb = half * 4 + blk
                    nc.scalar.activation(out=y2T[:, cb, :], in_=pss[blk],
                                         func=AF.Identity,
                                         bias=b2_sb[:, cb:cb + 1], scale=1.0)
            for tt in range(2):
                for cb in range(8):
                    tp5 = psA.tile([128, 128], f32r, name="tp5", tag="mm")
                    nc.tensor.transpose(tp5, y2T[:, cb, tt * 128:(tt + 1) * 128], ident)
                    nc.vector.tensor_add(out_sb[:, tt, cb * 128:(cb + 1) * 128],
                                         tp5, x1_sb[:, tt, cb * 128:(cb + 1) * 128])
            for tt in range(2):
                nc.sync.dma_start(out_ext[tt * 128:(tt + 1) * 128, :],
                                  out_sb[:, tt, :])

    nc.finalize()
    return nc


def _get_nc():
    if "nc" not in _CACHE:
        _CACHE["nc"] = _build_nc()
    return _CACHE["nc"]


def _tile_w(w, rows, cols):
    # [R, Cc] -> [R/128, Cc/512, 128, 512] contiguous tiles
    r, c = w.shape
    return np.ascontiguousarray(
        w.reshape(r // 128, 128, c // 512, 512).transpose(0, 2, 1, 3))


def _make_in_maps(inputs):
    import ml_dtypes
    bf = ml_dtypes.bfloat16
    x = np.asarray(inputs["x"], np.float32)
    mask = np.asarray(inputs["padding_mask"]).astype(bool)
    alibi = np.asarray(inputs["alibi_bias"], np.float32)
    wqkv = np.asarray(inputs["Wqkv"], np.float32)
    bqkv = np.asarray(inputs["bqkv"], np.float32)
    wproj = np.asarray(inputs["Wproj"], np.float32)
    bproj = np.asarray(inputs["bproj"], np.float32)
    w1 = np.asarray(inputs["W1"], np.float32)
    b1 = np.asarray(inputs["b1"], np.float32)
    w2 = np.asarray(inputs["W2"], np.float32)
    b2 = np.asarray(inputs["b2"], np.float32)
    ls = np.asarray(inputs["logit_scale"], np.float32).reshape(H)
    scale = np.exp(np.minimum(ls, math.log(100.0))).astype(np.float32)
    amax = float(alibi.max())
    bound = scale + amax + 1.0
    nbound = np.ascontiguousarray(np.tile((-bound).astype(np.float32)[None, :],
                                          (128, 1)))
    consts = np.ascontiguousarray(np.concatenate(
        [np.eye(128, dtype=np.float32),
         np.ones((128, 64), dtype=np.float32)], axis=1))

    common = {
        "wqkv_t": _tile_w(wqkv, C, 3 * C),
        "bqkv_t": np.ascontiguousarray(bqkv.reshape(24, 128).T),
        "wproj_t": _tile_w(wproj, C, C),
        "bproj_t": np.ascontiguousarray(bproj.reshape(8, 128).T),
        "w1_t": _tile_w(w1, C, HID),
        "b1_t": np.ascontiguousarray(b1.reshape(32, 128).T),
        "w2_t": _tile_w(w2, HID, C),
        "b2_t": np.ascontiguousarray(b2.reshape(8, 128).T),
        "ln1_g": np.asarray(inputs["ln1_g"], np.float32).reshape(1, C),
        "ln1_b": np.asarray(inputs["ln1_b"], np.float32).reshape(1, C),
        "ln2_g": np.asarray(inputs["ln2_g"], np.float32).reshape(1, C),
        "ln2_b": np.asarray(inputs["ln2_b"], np.float32).reshape(1, C),
        "scales": scale.reshape(1, H),
        "nbound": nbound,
        "consts": consts,
        "consts_bf": consts.astype(bf),
    }
    in_maps = []
    for c in range(NCORES):
        b, qi = divmod(c, GROUP)
        q0 = qi * TLOC
        alT = alibi[b, :, q0:q0 + TLOC, :].transpose(0, 2, 1)  # [H, N(k), TLOC]
        alT = alT + np.where(mask[b], np.float32(-1e9),
                             np.float32(0.0)).astype(np.float32)[None, :, None]
        alT = np.ascontiguousarray(
            alT.reshape(H, 8, 128, TLOC)).astype(bf)
        m = dict(common)
        m["x_loc"] = np.ascontiguousarray(x[b, q0:q0 + TLOC, :])
        m["alibi_t"] = alT
        in_maps.append(m)
    return in_maps


def _run(inputs, trace=False):
    from concourse import bass_utils
    nc = _get_nc()
    in_maps = _make_in_maps(inputs)
    res = bass_utils.run_bass_kernel_spmd(
        nc, in_maps, core_ids=list(range(NCORES)), trace=trace)
    outs = [np.asarray(res.results[c]["out"]) for c in range(NCORES)]
    y = np.stack(outs).reshape(B, GROUP * TLOC, C)
    return y.astype(np.float32), res


def kernel(**inputs):
    y, _ = _run(inputs, trace=False)
    return y


# revision 12
# speedup vs baseline: 1.5620x; 1.5065x over previous
"""Distributed Trainium2 kernel for nn_AltBlock (dense transformer block).

Sharding: sequence-parallel across 8 cores. Core c owns 256 query tokens of
batch c//4 (quarter (c%4) of the sequence). qkv/proj/mlp run per-core on the
local tokens with full (replicated) weights; attention needs all keys/values
of the batch, obtained with TWO small bf16 AllGathers (kn^T, then V) inside
each 4-core batch group, pipelined against S-matrix compute. Attention is
computed in the transposed layout S^T = [k_tokens(part), q_tokens(free)] so
the padding mask and softmax normalizer are partition-friendly, alibi is
added via an identity-matmul into PSUM, and P^T feeds the V-matmul directly.

Matmuls use float32r (f32 bits, full PE rate at free-dim>=256) on the
weight path and bfloat16 on the attention path. Weights are host-pre-tiled
into contiguous [128,512] DMA tiles.
"""

import math
import numpy as np
from contextlib import ExitStack

B, N, C, H = 2, 1024, 1024, 16
D = C // H          # 64
HID = 4 * C         # 4096
NCORES = 8
GROUP = 4           # cores per batch
TLOC = N // GROUP   # 256 local (query) tokens per core
EPS = 1e-5

_CACHE = {}


def _build_nc():
    import concourse.bass as bass
    import concourse.tile as tile
    from concourse import bacc, mybir

    f32 = mybir.dt.float32
    f32r = mybir.dt.float32r
    bf16 = mybir.dt.bfloat16
    AF = mybir.ActivationFunctionType
    OP = mybir.AluOpType

    nc = bacc.Bacc(None, target_bir_lowering=False)

    x_in = nc.dram_tensor("x_loc", [TLOC, C], f32, kind="ExternalInput")
    alibi_in = nc.dram_tensor("alibi_t", [H, 8, 128, TLOC], bf16, kind="ExternalInput")
    wqkv_in = nc.dram_tensor("wqkv_t", [8, 6, 128, 512], f32r, kind="ExternalInput")
    wproj_in = nc.dram_tensor("wproj_t", [8, 2, 128, 512], f32r, kind="ExternalInput")
    w1_in = nc.dram_tensor("w1_t", [8, 8, 128, 512], f32r, kind="ExternalInput")
    w2_in = nc.dram_tensor("w2_t", [32, 2, 128, 512], f32r, kind="ExternalInput")
    bqkv_in = nc.dram_tensor("bqkv_t", [128, 24], f32, kind="ExternalInput")
    bproj_in = nc.dram_tensor("bproj_t", [128, 8], f32, kind="ExternalInput")
    b1_in = nc.dram_tensor("b1_t", [128, 32], f32, kind="ExternalInput")
    b2_in = nc.dram_tensor("b2_t", [128, 8], f32, kind="ExternalInput")
    ln1g_in = nc.dram_tensor("ln1_g", [1, C], f32r, kind="ExternalInput")
    ln1b_in = nc.dram_tensor("ln1_b", [1, C], f32r, kind="ExternalInput")
    ln2g_in = nc.dram_tensor("ln2_g", [1, C], f32r, kind="ExternalInput")
    ln2b_in = nc.dram_tensor("ln2_b", [1, C], f32r, kind="ExternalInput")
    scales_in = nc.dram_tensor("scales", [2, 8, 256], f32, kind="ExternalInput")
    nbound_in = nc.dram_tensor("nbound", [128, H], f32, kind="ExternalInput")
    consts_in = nc.dram_tensor("consts", [128, 384], f32r, kind="ExternalInput")
    constsb_in = nc.dram_tensor("consts_bf", [128, 384], bf16, kind="ExternalInput")
    out_ext = nc.dram_tensor("out", [TLOC, C], f32, kind="ExternalOutput")

    def bcast_ap(handle):
        ap = handle[:]
        return bass.AP(tensor=ap.tensor, offset=ap.offset, ap=[[0, 128], [1, C]])

    with ExitStack() as stack:
        stack.enter_context(nc.allow_low_precision(reason="f32r views of f32"))
        tc = stack.enter_context(tile.TileContext(nc))
        pconst = stack.enter_context(tc.tile_pool(name="pconst", bufs=1))
        pdram = stack.enter_context(tc.tile_pool(name="pdram", bufs=1, space="DRAM"))
        psA = stack.enter_context(tc.tile_pool(name="psA", bufs=4, space="PSUM"))
        psS = stack.enter_context(tc.tile_pool(name="psS", bufs=2, space="PSUM"))

        # ---- constants / persistents ----
        consts_sb = pconst.tile([128, 384], f32r, name="consts_sb")
        nc.sync.dma_start(consts_sb, consts_in[:])
        ident = consts_sb[:, 0:128]
        sel_64 = consts_sb[:, 192:194]
        sel2T = consts_sb[0:2, 194:322]
        constsb_sb = pconst.tile([128, 384], bf16, name="constsb_sb")
        nc.sync.dma_start(constsb_sb, constsb_in[:])
        ident_bf = constsb_sb[:, 0:128]
        ones_128_64_bf = constsb_sb[:, 128:192]

        g1_sb = pconst.tile([128, C], f32r, name="g1_sb")
        nc.sync.dma_start(g1_sb, bcast_ap(ln1g_in))
        b1ln_sb = pconst.tile([128, C], f32r, name="b1ln_sb")
        nc.sync.dma_start(b1ln_sb, bcast_ap(ln1b_in))
        g2_sb = pconst.tile([128, C], f32r, name="g2_sb")
        nc.sync.dma_start(g2_sb, bcast_ap(ln2g_in))
        b2ln_sb = pconst.tile([128, C], f32r, name="b2ln_sb")
        nc.sync.dma_start(b2ln_sb, bcast_ap(ln2b_in))

        bqkv_sb = pconst.tile([128, 24], f32, name="bqkv_sb")
        nc.sync.dma_start(bqkv_sb, bqkv_in[:])
        bproj_sb = pconst.tile([128, 8], f32, name="bproj_sb")
        nc.sync.dma_start(bproj_sb, bproj_in[:])
        b1_sb = pconst.tile([128, 32], f32, name="b1_sb")
        nc.sync.dma_start(b1_sb, b1_in[:])
        b2_sb = pconst.tile([128, 8], f32, name="b2_sb")
        nc.sync.dma_start(b2_sb, b2_in[:])
        scales_sb = pconst.tile([2, 8, 256], f32, name="scales_sb")
        nc.sync.dma_start(scales_sb, scales_in[:])
        nbound_sb = pconst.tile([128, H], f32, name="nbound_sb")
        nc.sync.dma_start(nbound_sb, nbound_in[:])
        eps_sb = pconst.tile([128, 1], f32, name="eps_sb")
        nc.vector.memset(eps_sb, EPS)

        x_sb = pconst.tile([128, 2, C], f32, name="x_sb")
        for tt in range(2):
            nc.sync.dma_start(x_sb[:, tt, :], x_in[tt * 128:(tt + 1) * 128, :])
        x1_sb = pconst.tile([128, 2, C], f32, name="x1_sb")
        qnT = pconst.tile([128, 8, TLOC], bf16, name="qnT")
        OT_sb = pconst.tile([128, 8, TLOC], f32r, name="OT_sb")
        projT = pconst.tile([128, 8, TLOC], f32r, name="projT")
        out_sb = pconst.tile([128, 2, C], f32, name="out_sb")

        def layernorm(pool, x_slice, g_t, b_t, out_t):
            stats = pool.tile([128, 2, 6], f32, name="lnstats", tag="lnstats")
            for sg in range(2):
                nc.vector.bn_stats(out=stats[:, sg, :],
                                   in_=x_slice[:, sg * 512:(sg + 1) * 512])
            mv = pool.tile([128, 2], f32, name="lnmv", tag="lnmv")
            nc.vector.bn_aggr(out=mv, in_=stats)
            rstd = pool.tile([128, 1], f32, name="lnrstd", tag="lnrstd")
            nc.scalar.activation(out=rstd, in_=mv[:, 1:2], func=AF.Sqrt,
                                 bias=eps_sb[:, 0:1])
            nc.vector.reciprocal(out=rstd, in_=rstd)
            tmp = pool.tile([128, C], f32, name="lntmp", tag="lntmp", bufs=2)
            nc.vector.tensor_scalar(out=tmp, in0=x_slice, scalar1=mv[:, 0:1],
                                    scalar2=rstd, op0=OP.subtract, op1=OP.mult)
            nc.vector.tensor_mul(tmp, tmp, g_t)
            nc.vector.tensor_add(out_t, tmp, b_t)

        def qkv_super(sup, qkv_sb, hT):
            # one 512-col super-block of the qkv matmul, accumulated over C
            pss = [psA.tile([128, TLOC], f32, name=f"qps{blk}", tag="mm")
                   for blk in range(4)]
            for cc in range(8):
                wq = wstA.tile([128, 512], f32r, name="wq", tag="w")
                nc.sync.dma_start(wq, wqkv_in[cc, sup])
                for blk in range(4):
                    nc.tensor.matmul(pss[blk],
                                     lhsT=wq[:, blk * 128:(blk + 1) * 128],
                                     rhs=hT[:, cc, :],
                                     start=(cc == 0), stop=(cc == 7))
            for blk in range(4):
                cb = sup * 4 + blk
                nc.scalar.activation(out=qkv_sb[:, cb, :], in_=pss[blk],
                                     func=AF.Identity,
                                     bias=bqkv_sb[:, cb:cb + 1], scale=1.0)

        def norm_heads(pool, qkv_sb, src_col0, dst, with_scale):
            q2 = pool.tile([128, 8, TLOC], f32r, name="q2", tag="q2", bufs=1)
            nc.vector.tensor_mul(q2, qkv_sb[:, src_col0:src_col0 + 8, :],
                                 qkv_sb[:, src_col0:src_col0 + 8, :])
            rn_all = pool.tile([2, 8, TLOC], f32, name="rn_all", tag="rn", bufs=1)
            for blk in range(8):
                ssq = psS.tile([2, TLOC], f32, name="ssq", tag="acc")
                nc.tensor.matmul(ssq, lhsT=sel_64, rhs=q2[:, blk, :])
                nc.scalar.activation(out=rn_all[:, blk, :], in_=ssq, func=AF.Sqrt)
            rn_flat = rn_all.rearrange("p a b -> p (a b)")
            nc.vector.tensor_scalar(out=rn_flat, in0=rn_flat, scalar1=1e-12,
                                    scalar2=None, op0=OP.max)
            nc.vector.reciprocal(rn_flat, rn_flat)
            if with_scale:
                nc.vector.tensor_mul(rn_flat, rn_flat,
                                     scales_sb.rearrange("p a b -> p (a b)"))
            rnr = pool.tile([2, 8, TLOC], f32r, name="rnr", tag="rnr", bufs=1)
            nc.vector.tensor_copy(rnr.rearrange("p a b -> p (a b)"), rn_flat)
            for blk in range(8):
                bc = psS.tile([128, TLOC], f32, name="bc", tag="s")
                nc.tensor.matmul(bc, lhsT=sel2T, rhs=rnr[:, blk, :])
                nc.vector.tensor_mul(dst[:, blk, :], bc,
                                     qkv_sb[:, src_col0 + blk, :])

        # ============== Phase A: LN1, qkv (kv first), pack, AGs ==============
        bounce_kn = pdram.tile([128, 2048], bf16, name="bounce_kn")
        bounce_v = pdram.tile([128, 2048], bf16, name="bounce_v")
        ag_kn = pdram.tile([512, 2048], bf16, name="ag_kn")
        ag_v = pdram.tile([512, 2048], bf16, name="ag_v")
        with tc.tile_pool(name="pA", bufs=1) as pA, \
             tc.tile_pool(name="wstA", bufs=10) as wstA, \
             tc.tile_pool(name="ptmpA", bufs=4) as ptmpA:
            h_sb = pA.tile([128, 2, C], f32r, name="h_sb")
            for tt in range(2):
                layernorm(ptmpA, x_sb[:, tt, :], g1_sb, b1ln_sb, h_sb[:, tt, :])
            hT = pA.tile([128, 8, TLOC], f32r, name="hT")
            for tt in range(2):
                for cc in range(8):
                    tp = psA.tile([128, 128], f32r, name="tp", tag="mm")
                    nc.tensor.transpose(tp, h_sb[:, tt, cc * 128:(cc + 1) * 128], ident)
                    nc.scalar.activation(out=hT[:, cc, tt * 128:(tt + 1) * 128],
                                         in_=tp, func=AF.Copy)

            qkv_sb = pA.tile([128, 24, TLOC], f32r, name="qkv_sb")
            # K column supers first so AG1 can start as early as possible
            for sup in (2, 3):
                qkv_super(sup, qkv_sb, hT)
            knT_loc = pA.tile([128, 8, TLOC], bf16, name="knT_loc")
            norm_heads(ptmpA, qkv_sb, 8, knT_loc, with_scale=False)
            nc.sync.dma_start(bounce_kn, knT_loc.rearrange("p a b -> p (a b)"))
            nc.gpsimd.collective_compute(
                "AllGather", OP.bypass,
                ins=[bounce_kn.opt()], outs=[ag_kn.opt()],
                replica_groups=[[0, 1, 2, 3], [4, 5, 6, 7]],
            )
            for sup in (4, 5):
                qkv_super(sup, qkv_sb, hT)
            v_loc = pA.tile([128, 2, C], bf16, name="v_loc")
            for cb in range(8):
                for tt in range(2):
                    tp2 = psA.tile([128, 128], f32r, name="tp2", tag="mm")
                    nc.tensor.transpose(tp2, qkv_sb[:, 16 + cb, tt * 128:(tt + 1) * 128],
                                        ident)
                    nc.scalar.activation(out=v_loc[:, tt, cb * 128:(cb + 1) * 128],
                                         in_=tp2, func=AF.Copy)
            nc.sync.dma_start(bounce_v, v_loc.rearrange("p a b -> p (a b)"))
            nc.gpsimd.collective_compute(
                "AllGather", OP.bypass,
                ins=[bounce_v.opt()], outs=[ag_v.opt()],
                replica_groups=[[0, 1, 2, 3], [4, 5, 6, 7]],
            )
            # Q column supers + q normalization (overlap the collectives)
            for sup in (0, 1):
                qkv_super(sup, qkv_sb, hT)
            norm_heads(ptmpA, qkv_sb, 0, qnT, with_scale=True)

        # ============== Phase B: attention (pipelined over AG2), proj =========
        with tc.tile_pool(name="pB", bufs=1) as pB, \
             tc.tile_pool(name="alst", bufs=6) as alst, \
             tc.tile_pool(name="wstB", bufs=6) as wstB, \
             tc.tile_pool(name="ptmpB", bufs=4) as ptmpB:
            kn_r, v_r = [], []
            for r in range(4):
                t = pB.tile([128, 2048], bf16, name=f"kn{r}", tag=f"kn{r}")
                nc.sync.dma_start(t, ag_kn[r * 128:(r + 1) * 128, :])
                kn_r.append(t)
            for r in range(4):
                t = pB.tile([128, 2048], bf16, name=f"v{r}", tag=f"v{r}")
                nc.sync.dma_start(t, ag_v[r * 128:(r + 1) * 128, :])
                v_r.append(t)

            P_T = pB.tile([128, H, 8, TLOC], bf16, name="P_T")
            rs_sb = pB.tile([64, H, TLOC], bf16, name="rs_sb")
            # pass 1: S = kn^T q * scale + alibi (+mask), exp, row-sums
            for h in range(H):
                rows = slice(64 * (h % 2), 64 * (h % 2) + 64)
                sums_ps = psS.tile([64, TLOC], f32, name="sums_ps", tag="acc")
                for kc in range(8):
                    r, tt = kc // 2, kc % 2
                    al = alst.tile([128, TLOC], bf16, name="al", tag="al")
                    nc.sync.dma_start(al, alibi_in[h, kc])
                    S = psS.tile([128, TLOC], f32, name="S", tag="s")
                    knT_sl = kn_r[r][rows, (h // 2) * 256 + tt * 128:
                                     (h // 2) * 256 + tt * 128 + 128]
                    nc.tensor.matmul(S, lhsT=knT_sl, rhs=qnT[rows, h // 2, :],
                                     start=True, stop=False)
                    nc.tensor.matmul(S, lhsT=ident_bf, rhs=al,
                                     start=False, stop=True)
                    nc.scalar.activation(out=P_T[:, h, kc, :], in_=S, func=AF.Exp,
                                         bias=nbound_sb[:, h:h + 1], scale=1.0)
                    nc.tensor.matmul(sums_ps, lhsT=ones_128_64_bf,
                                     rhs=P_T[:, h, kc, :],
                                     start=(kc == 0), stop=(kc == 7))
                nc.vector.reciprocal(rs_sb[:, h, :], sums_ps)
            # pass 2: O^T = V^T P^T (waits on AG2), normalize by row-sums
            for h in range(H):
                rows = slice(64 * (h % 2), 64 * (h % 2) + 64)
                OT_ps = psS.tile([64, TLOC], f32, name="OT_ps", tag="acc")
                for kc in range(8):
                    r, tt = kc // 2, kc % 2
                    v_sl = v_r[r][:, tt * 1024 + h * 64: tt * 1024 + h * 64 + 64]
                    nc.tensor.matmul(OT_ps, lhsT=v_sl, rhs=P_T[:, h, kc, :],
                                     start=(kc == 0), stop=(kc == 7))
                nc.vector.tensor_mul(OT_sb[rows, h // 2, :], OT_ps, rs_sb[:, h, :])

            # proj
            for sup in range(2):
                pss = [psA.tile([128, TLOC], f32, name=f"pps{blk}", tag="mm")
                       for blk in range(4)]
                for cc in range(8):
                    wp = wstB.tile([128, 512], f32r, name="wp", tag="w")
                    nc.sync.dma_start(wp, wproj_in[cc, sup])
                    for blk in range(4):
                        nc.tensor.matmul(pss[blk],
                                         lhsT=wp[:, blk * 128:(blk + 1) * 128],
                                         rhs=OT_sb[:, cc, :],
                                         start=(cc == 0), stop=(cc == 7))
                for blk in range(4):
                    cb = sup * 4 + blk
                    nc.scalar.activation(out=projT[:, cb, :], in_=pss[blk],
                                         func=AF.Identity,
                                         bias=bproj_sb[:, cb:cb + 1], scale=1.0)
            for tt in range(2):
                for cb in range(8):
                    tp3 = psA.tile([128, 128], f32r, name="tp3", tag="mm")
                    nc.tensor.transpose(tp3, projT[:, cb, tt * 128:(tt + 1) * 128],
                                        ident)
                    nc.vector.tensor_add(x1_sb[:, tt, cb * 128:(cb + 1) * 128],
                                         tp3, x_sb[:, tt, cb * 128:(cb + 1) * 128])

        # ================= Phase C: LN2 + MLP =================
        with tc.tile_pool(name="pC", bufs=1) as pC, \
             tc.tile_pool(name="wstC", bufs=10) as wstC, \
             tc.tile_pool(name="ptmpC", bufs=4) as ptmpC:
            y_sb = pC.tile([128, 2, C], f32r, name="y_sb")
            for tt in range(2):
                layernorm(ptmpC, x1_sb[:, tt, :], g2_sb, b2ln_sb, y_sb[:, tt, :])
            yT = pC.tile([128, 8, TLOC], f32r, name="yT")
            for tt in range(2):
                for cc in range(8):
                    tp4 = psA.tile([128, 128], f32r, name="tp4", tag="mm")
                    nc.tensor.transpose(tp4, y_sb[:, tt, cc * 128:(cc + 1) * 128], ident)
                    nc.scalar.activation(out=yT[:, cc, tt * 128:(tt + 1) * 128],
                                         in_=tp4, func=AF.Copy)

            h1 = pC.tile([128, 32, TLOC], f32r, name="h1")
            for sup in range(8):
                pss = [psA.tile([128, TLOC], f32, name=f"m1ps{blk}", tag="mm")
                       for blk in range(4)]
                for cc in range(8):
                    w1t = wstC.tile([128, 512], f32r, name="w1t", tag="w")
                    nc.sync.dma_start(w1t, w1_in[cc, sup])
                    for blk in range(4):
                        nc.tensor.matmul(pss[blk],
                                         lhsT=w1t[:, blk * 128:(blk + 1) * 128],
                                         rhs=yT[:, cc, :],
                                         start=(cc == 0), stop=(cc == 7))
                for blk in range(4):
                    hb = sup * 4 + blk
                    nc.scalar.activation(out=h1[:, hb, :], in_=pss[blk],
                                         func=AF.Gelu,
                                         bias=b1_sb[:, hb:hb + 1], scale=1.0)

            y2T = pC.tile([128, 8, TLOC], f32r, name="y2T")
            for half in range(2):
                pss = [psA.tile([128, TLOC], f32, name=f"m2ps{blk}", tag="mm")
                       for blk in range(4)]
                for hc in range(32):
                    w2t = wstC.tile([128, 512], f32r, name="w2t", tag="w")
                    nc.sync.dma_start(w2t, w2_in[hc, half])
                    for blk in range(4):
                        nc.tensor.matmul(pss[blk],
                                         lhsT=w2t[:, blk * 128:(blk + 1) * 128],
                                         rhs=h1[:, hc, :],
                                         start=(hc == 0), stop=(hc == 31))
                for blk in range(4):
                    cb = half * 4 + blk
                    nc.scalar.activation(out=y2T[:, cb, :], in_=pss[blk],
                                         func=AF.Identity,
                                         bias=b2_sb[:, cb:cb + 1], scale=1.0)
            for tt in range(2):
                for cb in range(8):
                    tp5 = psA.tile([128, 128], f32r, name="tp5", tag="mm")
                    nc.tensor.transpose(tp5, y2T[:, cb, tt * 128:(tt + 1) * 128], ident)
                    nc.vector.tensor_add(out_sb[:, tt, cb * 128:(cb + 1) * 128],
                                         tp5, x1_sb[:, tt, cb * 128:(cb + 1) * 128])
            for tt in range(2):
                nc.sync.dma_start(out_ext[tt * 128:(tt + 1) * 128, :],
                                  out_sb[:, tt, :])

    nc.finalize()
    return nc


def _get_nc():
    if "nc" not in _CACHE:
        _CACHE["nc"] = _build_nc()
    return _CACHE["nc"]


def _tile_w(w, rows, cols):
    # [R, Cc] -> [R/128, Cc/512, 128, 512] contiguous tiles
    r, c = w.shape
    return np.ascontiguousarray(
        w.reshape(r // 128, 128, c // 512, 512).transpose(0, 2, 1, 3))


def _make_in_maps(inputs):
    import ml_dtypes
    bf = ml_dtypes.bfloat16
    x = np.asarray(inputs["x"], np.float32)
    mask = np.asarray(inputs["padding_mask"]).astype(bool)
    alibi = np.asarray(inputs["alibi_bias"], np.float32)
    wqkv = np.asarray(inputs["Wqkv"], np.float32)
    bqkv = np.asarray(inputs["bqkv"], np.float32)
    wproj = np.asarray(inputs["Wproj"], np.float32)
    bproj = np.asarray(inputs["bproj"], np.float32)
    w1 = np.asarray(inputs["W1"], np.float32)
    b1 = np.asarray(inputs["b1"], np.float32)
    w2 = np.asarray(inputs["W2"], np.float32)
    b2 = np.asarray(inputs["b2"], np.float32)
    ls = np.asarray(inputs["logit_scale"], np.float32).reshape(H)
    scale = np.exp(np.minimum(ls, math.log(100.0))).astype(np.float32)
    amax = float(alibi.max())
    bound = scale + amax + 1.0
    nbound = np.ascontiguousarray(np.tile((-bound).astype(np.float32)[None, :],
                                          (128, 1)))
    consts = np.zeros((128, 384), dtype=np.float32)
    consts[:, 0:128] = np.eye(128, dtype=np.float32)
    consts[:, 128:192] = 1.0
    consts[0:64, 192] = 1.0
    consts[64:128, 193] = 1.0
    consts[0, 194:258] = 1.0
    consts[1, 258:322] = 1.0
    consts = np.ascontiguousarray(consts)
    scales_bc = np.zeros((2, 8, 256), dtype=np.float32)
    for h in range(H):
        scales_bc[h % 2, h // 2, :] = scale[h]

    common = {
        "wqkv_t": _tile_w(wqkv, C, 3 * C),
        "bqkv_t": np.ascontiguousarray(bqkv.reshape(24, 128).T),
        "wproj_t": _tile_w(wproj, C, C),
        "bproj_t": np.ascontiguousarray(bproj.reshape(8, 128).T),
        "w1_t": _tile_w(w1, C, HID),
        "b1_t": np.ascontiguousarray(b1.reshape(32, 128).T),
        "w2_t": _tile_w(w2, HID, C),
        "b2_t": np.ascontiguousarray(b2.reshape(8, 128).T),
        "ln1_g": np.asarray(inputs["ln1_g"], np.float32).reshape(1, C),
        "ln1_b": np.asarray(inputs["ln1_b"], np.float32).reshape(1, C),
        "ln2_g": np.asarray(inputs["ln2_g"], np.float32).reshape(1, C),
        "ln2_b": np.asarray(inputs["ln2_b"], np.float32).reshape(1, C),
        "scales": scales_bc,
        "nbound": nbound,
        "consts": consts,
        "consts_bf": consts.astype(bf),
    }
    in_maps = []
    for c in range(NCORES):
        b, qi = divmod(c, GROUP)
        q0 = qi * TLOC
        alT = alibi[b, :, q0:q0 + TLOC, :].transpose(0, 2, 1)  # [H, N(k), TLOC]
        alT = alT + np.where(mask[b], np.float32(-1e9),
                             np.float32(0.0)).astype(np.float32)[None, :, None]
        alT = np.ascontiguousarray(
            alT.reshape(H, 8, 128, TLOC)).astype(bf)
        m = dict(common)
        m["x_loc"] = np.ascontiguousarray(x[b, q0:q0 + TLOC, :])
        m["alibi_t"] = alT
        in_maps.append(m)
    return in_maps


def _run(inputs, trace=False):
    from concourse import bass_utils
    nc = _get_nc()
    in_maps = _make_in_maps(inputs)
    res = bass_utils.run_bass_kernel_spmd(
        nc, in_maps, core_ids=list(range(NCORES)), trace=trace)
    outs = [np.asarray(res.results[c]["out"]) for c in range(NCORES)]
    y = np.stack(outs).reshape(B, GROUP * TLOC, C)
    return y.astype(np.float32), res


def kernel(**inputs):
    y, _ = _run(inputs, trace=False)
    return y


# revision 14
# speedup vs baseline: 1.7103x; 1.0950x over previous
"""Distributed Trainium2 kernel for nn_AltBlock (dense transformer block).

Sharding: sequence-parallel across 8 cores. Core c owns 256 query tokens of
batch c//4 (quarter (c%4) of the sequence). qkv/proj/mlp run per-core on the
local tokens with full (replicated) weights; attention needs all keys/values
of the batch, obtained with TWO small bf16 AllGathers (kn^T, then V) inside
each 4-core batch group, pipelined against S-matrix compute. Attention is
computed in the transposed layout S^T = [k_tokens(part), q_tokens(free)] so
the padding mask and softmax normalizer are partition-friendly, alibi is
added via an identity-matmul into PSUM, and P^T feeds the V-matmul directly.

Matmuls use float32r (f32 bits, full PE rate at free-dim>=256) on the
weight path and bfloat16 on the attention path. Weights are host-pre-tiled
into contiguous [128,512] DMA tiles.
"""

import math
import numpy as np
from contextlib import ExitStack

B, N, C, H = 2, 1024, 1024, 16
D = C // H          # 64
HID = 4 * C         # 4096
NCORES = 8
GROUP = 4           # cores per batch
TLOC = N // GROUP   # 256 local (query) tokens per core
EPS = 1e-5

_CACHE = {}


def _build_nc():
    import concourse.bass as bass
    import concourse.tile as tile
    from concourse import bacc, mybir

    f32 = mybir.dt.float32
    f32r = mybir.dt.float32r
    bf16 = mybir.dt.bfloat16
    AF = mybir.ActivationFunctionType
    OP = mybir.AluOpType

    nc = bacc.Bacc(None, target_bir_lowering=False)

    x_in = nc.dram_tensor("x_loc", [TLOC, C], f32, kind="ExternalInput")
    alibi_in = nc.dram_tensor("alibi_t", [H, 8, 128, TLOC], bf16, kind="ExternalInput")
    wqkv_in = nc.dram_tensor("wqkv_t", [8, 6, 128, 512], bf16, kind="ExternalInput")
    wproj_in = nc.dram_tensor("wproj_t", [8, 2, 128, 512], bf16, kind="ExternalInput")
    w1_in = nc.dram_tensor("w1_t", [8, 8, 128, 512], bf16, kind="ExternalInput")
    w2_in = nc.dram_tensor("w2_t", [32, 2, 128, 512], bf16, kind="ExternalInput")
    bqkv_in = nc.dram_tensor("bqkv_t", [128, 24], f32, kind="ExternalInput")
    bproj_in = nc.dram_tensor("bproj_t", [128, 8], f32, kind="ExternalInput")
    b1_in = nc.dram_tensor("b1_t", [128, 32], f32, kind="ExternalInput")
    b2_in = nc.dram_tensor("b2_t", [128, 8], f32, kind="ExternalInput")
    ln1g_in = nc.dram_tensor("ln1_g", [1, C], f32r, kind="ExternalInput")
    ln1b_in = nc.dram_tensor("ln1_b", [1, C], f32r, kind="ExternalInput")
    ln2g_in = nc.dram_tensor("ln2_g", [1, C], f32r, kind="ExternalInput")
    ln2b_in = nc.dram_tensor("ln2_b", [1, C], f32r, kind="ExternalInput")
    scales_in = nc.dram_tensor("scales", [2, 8, 256], f32, kind="ExternalInput")
    nbound_in = nc.dram_tensor("nbound", [128, H], f32, kind="ExternalInput")
    consts_in = nc.dram_tensor("consts", [128, 384], f32r, kind="ExternalInput")
    constsb_in = nc.dram_tensor("consts_bf", [128, 384], bf16, kind="ExternalInput")
    out_ext = nc.dram_tensor("out", [TLOC, C], f32, kind="ExternalOutput")

    def bcast_ap(handle):
        ap = handle[:]
        return bass.AP(tensor=ap.tensor, offset=ap.offset, ap=[[0, 128], [1, C]])

    with ExitStack() as stack:
        stack.enter_context(nc.allow_low_precision(reason="f32r views of f32"))
        tc = stack.enter_context(tile.TileContext(nc))
        pconst = stack.enter_context(tc.tile_pool(name="pconst", bufs=1))
        pdram = stack.enter_context(tc.tile_pool(name="pdram", bufs=1, space="DRAM"))
        psA = stack.enter_context(tc.tile_pool(name="psA", bufs=4, space="PSUM"))
        psS = stack.enter_context(tc.tile_pool(name="psS", bufs=2, space="PSUM"))

        # ---- constants / persistents ----
        constsb_sb = pconst.tile([128, 384], bf16, name="constsb_sb")
        nc.sync.dma_start(constsb_sb, constsb_in[:])
        ident = constsb_sb[:, 0:128]
        sel_64 = constsb_sb[:, 192:194]
        sel2T = constsb_sb[0:2, 194:322]
        ident_bf = ident
        ones_128_64_bf = constsb_sb[:, 128:192]

        g1_sb = pconst.tile([128, C], f32r, name="g1_sb")
        nc.sync.dma_start(g1_sb, bcast_ap(ln1g_in))
        b1ln_sb = pconst.tile([128, C], f32r, name="b1ln_sb")
        nc.sync.dma_start(b1ln_sb, bcast_ap(ln1b_in))
        g2_sb = pconst.tile([128, C], f32r, name="g2_sb")
        nc.sync.dma_start(g2_sb, bcast_ap(ln2g_in))
        b2ln_sb = pconst.tile([128, C], f32r, name="b2ln_sb")
        nc.sync.dma_start(b2ln_sb, bcast_ap(ln2b_in))

        bqkv_sb = pconst.tile([128, 24], f32, name="bqkv_sb")
        nc.sync.dma_start(bqkv_sb, bqkv_in[:])
        bproj_sb = pconst.tile([128, 8], f32, name="bproj_sb")
        nc.sync.dma_start(bproj_sb, bproj_in[:])
        b1_sb = pconst.tile([128, 32], f32, name="b1_sb")
        nc.sync.dma_start(b1_sb, b1_in[:])
        b2_sb = pconst.tile([128, 8], f32, name="b2_sb")
        nc.sync.dma_start(b2_sb, b2_in[:])
        scales_sb = pconst.tile([2, 8, 256], f32, name="scales_sb")
        nc.sync.dma_start(scales_sb, scales_in[:])
        nbound_sb = pconst.tile([128, H], f32, name="nbound_sb")
        nc.sync.dma_start(nbound_sb, nbound_in[:])
        eps_sb = pconst.tile([128, 1], f32, name="eps_sb")
        nc.vector.memset(eps_sb, EPS)

        x_sb = pconst.tile([128, 2, C], f32, name="x_sb")
        for tt in range(2):
            nc.sync.dma_start(x_sb[:, tt, :], x_in[tt * 128:(tt + 1) * 128, :])
        x1_sb = pconst.tile([128, 2, C], f32, name="x1_sb")
        qnT = pconst.tile([128, 8, TLOC], bf16, name="qnT")
        OT_sb = pconst.tile([128, 8, TLOC], bf16, name="OT_sb")
        projT = pconst.tile([128, 8, TLOC], bf16, name="projT")
        out_sb = pconst.tile([128, 2, C], f32, name="out_sb")

        def layernorm(pool, x_slice, g_t, b_t, out_t):
            stats = pool.tile([128, 2, 6], f32, name="lnstats", tag="lnstats")
            for sg in range(2):
                nc.vector.bn_stats(out=stats[:, sg, :],
                                   in_=x_slice[:, sg * 512:(sg + 1) * 512])
            mv = pool.tile([128, 2], f32, name="lnmv", tag="lnmv")
            nc.vector.bn_aggr(out=mv, in_=stats)
            rstd = pool.tile([128, 1], f32, name="lnrstd", tag="lnrstd")
            nc.scalar.activation(out=rstd, in_=mv[:, 1:2], func=AF.Sqrt,
                                 bias=eps_sb[:, 0:1])
            nc.vector.reciprocal(out=rstd, in_=rstd)
            tmp = pool.tile([128, C], f32, name="lntmp", tag="lntmp", bufs=2)
            nc.vector.tensor_scalar(out=tmp, in0=x_slice, scalar1=mv[:, 0:1],
                                    scalar2=rstd, op0=OP.subtract, op1=OP.mult)
            nc.vector.tensor_mul(tmp, tmp, g_t)
            nc.vector.tensor_add(out_t, tmp, b_t)

        def qkv_super(sup, qkv_sb, hT):
            # one 512-col super-block of the qkv matmul, accumulated over C
            pss = [psA.tile([128, TLOC], f32, name=f"qps{blk}", tag="mm")
                   for blk in range(4)]
            for cc in range(8):
                wq = wstA.tile([128, 512], bf16, name="wq", tag="w")
                nc.sync.dma_start(wq, wqkv_in[cc, sup])
                for blk in range(4):
                    nc.tensor.matmul(pss[blk],
                                     lhsT=wq[:, blk * 128:(blk + 1) * 128],
                                     rhs=hT[:, cc, :],
                                     start=(cc == 0), stop=(cc == 7))
            for blk in range(4):
                cb = sup * 4 + blk
                nc.scalar.activation(out=qkv_sb[:, cb, :], in_=pss[blk],
                                     func=AF.Identity,
                                     bias=bqkv_sb[:, cb:cb + 1], scale=1.0)

        def norm_heads(pool, qkv_sb, src_col0, dst, with_scale):
            q2 = pool.tile([128, 8, TLOC], bf16, name="q2", tag="q2", bufs=1)
            nc.vector.tensor_mul(q2, qkv_sb[:, src_col0:src_col0 + 8, :],
                                 qkv_sb[:, src_col0:src_col0 + 8, :])
            rn_all = pool.tile([2, 8, TLOC], f32, name="rn_all", tag="rn", bufs=1)
            for blk in range(8):
                ssq = psS.tile([2, TLOC], f32, name="ssq", tag="acc")
                nc.tensor.matmul(ssq, lhsT=sel_64, rhs=q2[:, blk, :])
                nc.scalar.activation(out=rn_all[:, blk, :], in_=ssq, func=AF.Sqrt)
            rn_flat = rn_all.rearrange("p a b -> p (a b)")
            nc.vector.tensor_scalar(out=rn_flat, in0=rn_flat, scalar1=1e-12,
                                    scalar2=None, op0=OP.max)
            nc.vector.reciprocal(rn_flat, rn_flat)
            if with_scale:
                nc.vector.tensor_mul(rn_flat, rn_flat,
                                     scales_sb.rearrange("p a b -> p (a b)"))
            rnr = pool.tile([2, 8, TLOC], bf16, name="rnr", tag="rnr", bufs=1)
            nc.vector.tensor_copy(rnr.rearrange("p a b -> p (a b)"), rn_flat)
            for blk in range(8):
                bc = psS.tile([128, TLOC], f32, name="bc", tag="s")
                nc.tensor.matmul(bc, lhsT=sel2T, rhs=rnr[:, blk, :])
                nc.vector.tensor_mul(dst[:, blk, :], bc,
                                     qkv_sb[:, src_col0 + blk, :])

        # ============== Phase A: LN1, qkv (kv first), pack, AGs ==============
        bounce_kn = pdram.tile([128, 2048], bf16, name="bounce_kn")
        bounce_v = pdram.tile([128, 2048], bf16, name="bounce_v")
        ag_kn = pdram.tile([512, 2048], bf16, name="ag_kn")
        ag_v = pdram.tile([512, 2048], bf16, name="ag_v")
        with tc.tile_pool(name="pA", bufs=1) as pA, \
             tc.tile_pool(name="wstA", bufs=10) as wstA, \
             tc.tile_pool(name="ptmpA", bufs=4) as ptmpA:
            h_sb = pA.tile([128, 2, C], bf16, name="h_sb")
            for tt in range(2):
                layernorm(ptmpA, x_sb[:, tt, :], g1_sb, b1ln_sb, h_sb[:, tt, :])
            hT = pA.tile([128, 8, TLOC], bf16, name="hT")
            for tt in range(2):
                for cc in range(8):
                    tp = psA.tile([128, 128], bf16, name="tp", tag="mm")
                    nc.tensor.transpose(tp, h_sb[:, tt, cc * 128:(cc + 1) * 128], ident)
                    nc.scalar.activation(out=hT[:, cc, tt * 128:(tt + 1) * 128],
                                         in_=tp, func=AF.Copy)

            qkv_sb = pA.tile([128, 24, TLOC], bf16, name="qkv_sb")
            # K column supers first so AG1 can start as early as possible
            for sup in (2, 3):
                qkv_super(sup, qkv_sb, hT)
            knT_loc = pA.tile([128, 8, TLOC], bf16, name="knT_loc")
            norm_heads(ptmpA, qkv_sb, 8, knT_loc, with_scale=False)
            nc.sync.dma_start(bounce_kn, knT_loc.rearrange("p a b -> p (a b)"))
            nc.gpsimd.collective_compute(
                "AllGather", OP.bypass,
                ins=[bounce_kn.opt()], outs=[ag_kn.opt()],
                replica_groups=[[0, 1, 2, 3], [4, 5, 6, 7]],
            )
            for sup in (4, 5):
                qkv_super(sup, qkv_sb, hT)
            v_loc = pA.tile([128, 2, C], bf16, name="v_loc")
            for cb in range(8):
                for tt in range(2):
                    tp2 = psA.tile([128, 128], bf16, name="tp2", tag="mm")
                    nc.tensor.transpose(tp2, qkv_sb[:, 16 + cb, tt * 128:(tt + 1) * 128],
                                        ident)
                    nc.scalar.activation(out=v_loc[:, tt, cb * 128:(cb + 1) * 128],
                                         in_=tp2, func=AF.Copy)
            nc.sync.dma_start(bounce_v, v_loc.rearrange("p a b -> p (a b)"))
            nc.gpsimd.collective_compute(
                "AllGather", OP.bypass,
                ins=[bounce_v.opt()], outs=[ag_v.opt()],
                replica_groups=[[0, 1, 2, 3], [4, 5, 6, 7]],
            )
            # Q column supers + q normalization (overlap the collectives)
            for sup in (0, 1):
                qkv_super(sup, qkv_sb, hT)
            norm_heads(ptmpA, qkv_sb, 0, qnT, with_scale=True)

        # ============== Phase B: attention (pipelined over AG2), proj =========
        with tc.tile_pool(name="pB", bufs=1) as pB, \
             tc.tile_pool(name="alst", bufs=6) as alst, \
             tc.tile_pool(name="wstB", bufs=6) as wstB, \
             tc.tile_pool(name="ptmpB", bufs=4) as ptmpB:
            kn_r, v_r = [], []
            for r in range(4):
                t = pB.tile([128, 2048], bf16, name=f"kn{r}", tag=f"kn{r}")
                nc.sync.dma_start(t, ag_kn[r * 128:(r + 1) * 128, :])
                kn_r.append(t)
            for r in range(4):
                t = pB.tile([128, 2048], bf16, name=f"v{r}", tag=f"v{r}")
                nc.sync.dma_start(t, ag_v[r * 128:(r + 1) * 128, :])
                v_r.append(t)

            P_T = pB.tile([128, H, 8, TLOC], bf16, name="P_T")
            rs_sb = pB.tile([64, H, TLOC], bf16, name="rs_sb")
            # pass 1: S = kn^T q * scale + alibi (+mask), exp, row-sums
            for h in range(H):
                rows = slice(64 * (h % 2), 64 * (h % 2) + 64)
                sums_ps = psS.tile([64, TLOC], f32, name="sums_ps", tag="acc")
                for pr in range(4):
                    al = alst.tile([128, 2, TLOC], bf16, name="al", tag="al")
                    nc.sync.dma_start(
                        al, alibi_in[h, 2 * pr:2 * pr + 2].rearrange("a p b -> p a b"))
                    S = psS.tile([128, 2, TLOC], f32, name="S", tag="s")
                    for j in range(2):
                        kc = 2 * pr + j
                        r, tt = kc // 2, kc % 2
                        knT_sl = kn_r[r][rows, (h // 2) * 256 + tt * 128:
                                         (h // 2) * 256 + tt * 128 + 128]
                        nc.tensor.matmul(S[:, j, :], lhsT=knT_sl,
                                         rhs=qnT[rows, h // 2, :],
                                         start=True, stop=False)
                        nc.tensor.matmul(S[:, j, :], lhsT=ident_bf, rhs=al[:, j, :],
                                         start=False, stop=True)
                    nc.scalar.activation(out=P_T[:, h, 2 * pr:2 * pr + 2, :], in_=S,
                                         func=AF.Exp,
                                         bias=nbound_sb[:, h:h + 1], scale=1.0)
                    for j in range(2):
                        kc = 2 * pr + j
                        nc.tensor.matmul(sums_ps, lhsT=ones_128_64_bf,
                                         rhs=P_T[:, h, kc, :],
                                         start=(kc == 0), stop=(kc == 7))
                nc.vector.reciprocal(rs_sb[:, h, :], sums_ps)
            # pass 2: O^T = V^T P^T (waits on AG2), normalize by row-sums
            for h in range(H):
                rows = slice(64 * (h % 2), 64 * (h % 2) + 64)
                OT_ps = psS.tile([64, TLOC], f32, name="OT_ps", tag="acc")
                for kc in range(8):
                    r, tt = kc // 2, kc % 2
                    v_sl = v_r[r][:, tt * 1024 + h * 64: tt * 1024 + h * 64 + 64]
                    nc.tensor.matmul(OT_ps, lhsT=v_sl, rhs=P_T[:, h, kc, :],
                                     start=(kc == 0), stop=(kc == 7))
                nc.vector.tensor_mul(OT_sb[rows, h // 2, :], OT_ps, rs_sb[:, h, :])

            # proj
            for sup in range(2):
                pss = [psA.tile([128, TLOC], f32, name=f"pps{blk}", tag="mm")
                       for blk in range(4)]
                for cc in range(8):
                    wp = wstB.tile([128, 512], bf16, name="wp", tag="w")
                    nc.sync.dma_start(wp, wproj_in[cc, sup])
                    for blk in range(4):
                        nc.tensor.matmul(pss[blk],
                                         lhsT=wp[:, blk * 128:(blk + 1) * 128],
                                         rhs=OT_sb[:, cc, :],
                                         start=(cc == 0), stop=(cc == 7))
                for blk in range(4):
                    cb = sup * 4 + blk
                    nc.scalar.activation(out=projT[:, cb, :], in_=pss[blk],
                                         func=AF.Identity,
                                         bias=bproj_sb[:, cb:cb + 1], scale=1.0)
            for tt in range(2):
                for cb in range(8):
                    tp3 = psA.tile([128, 128], bf16, name="tp3", tag="mm")
                    nc.tensor.transpose(tp3, projT[:, cb, tt * 128:(tt + 1) * 128],
                                        ident)
                    nc.vector.tensor_add(x1_sb[:, tt, cb * 128:(cb + 1) * 128],
                                         tp3, x_sb[:, tt, cb * 128:(cb + 1) * 128])

        # ================= Phase C: LN2 + MLP =================
        with tc.tile_pool(name="pC", bufs=1) as pC, \
             tc.tile_pool(name="wstC", bufs=10) as wstC, \
             tc.tile_pool(name="ptmpC", bufs=4) as ptmpC:
            y_sb = pC.tile([128, 2, C], bf16, name="y_sb")
            for tt in range(2):
                layernorm(ptmpC, x1_sb[:, tt, :], g2_sb, b2ln_sb, y_sb[:, tt, :])
            yT = pC.tile([128, 8, TLOC], bf16, name="yT")
            for tt in range(2):
                for cc in range(8):
                    tp4 = psA.tile([128, 128], bf16, name="tp4", tag="mm")
                    nc.tensor.transpose(tp4, y_sb[:, tt, cc * 128:(cc + 1) * 128], ident)
                    nc.vector.tensor_copy(yT[:, cc, tt * 128:(tt + 1) * 128], tp4)

            h1 = pC.tile([128, 32, TLOC], bf16, name="h1")
            for sup in range(8):
                pss = [psA.tile([128, TLOC], f32, name=f"m1ps{blk}", tag="mm")
                       for blk in range(4)]
                for cc in range(8):
                    w1t = wstC.tile([128, 512], bf16, name="w1t", tag="w")
                    nc.sync.dma_start(w1t, w1_in[cc, sup])
                    for blk in range(4):
                        nc.tensor.matmul(pss[blk],
                                         lhsT=w1t[:, blk * 128:(blk + 1) * 128],
                                         rhs=yT[:, cc, :],
                                         start=(cc == 0), stop=(cc == 7))
                for blk in range(4):
                    hb = sup * 4 + blk
                    nc.scalar.activation(out=h1[:, hb, :], in_=pss[blk],
                                         func=AF.Gelu,
                                         bias=b1_sb[:, hb:hb + 1], scale=1.0)

            y2T = pC.tile([128, 8, TLOC], bf16, name="y2T")
            for half in range(2):
                pss = [psA.tile([128, TLOC], f32, name=f"m2ps{blk}", tag="mm")
                       for blk in range(4)]
                for hc in range(32):
                    w2t = wstC.tile([128, 512], bf16, name="w2t", tag="w")
                    nc.sync.dma_start(w2t, w2_in[hc, half])
                    for blk in range(4):
                        nc.tensor.matmul(pss[blk],
                                         lhsT=w2t[:, blk * 128:(blk + 1) * 128],
                                         rhs=h1[:, hc, :],
                                         start=(hc == 0), stop=(hc == 31))
                for blk in range(4):
                    cb = half * 4 + blk
                    nc.vector.tensor_scalar(out=y2T[:, cb, :], in0=pss[blk],
                                            scalar1=b2_sb[:, cb:cb + 1],
                                            scalar2=None, op0=OP.add)
            for tt in range(2):
                for cb in range(8):
                    tp5 = psA.tile([128, 128], bf16, name="tp5", tag="mm")
                    nc.tensor.transpose(tp5, y2T[:, cb, tt * 128:(tt + 1) * 128], ident)
                    nc.vector.tensor_add(out_sb[:, tt, cb * 128:(cb + 1) * 128],
                                         tp5, x1_sb[:, tt, cb * 128:(cb + 1) * 128])
            for tt in range(2):
                nc.sync.dma_start(out_ext[tt * 128:(tt + 1) * 128, :],
                                  out_sb[:, tt, :])

    nc.finalize()
    return nc


def _get_nc():
    if "nc" not in _CACHE:
        _CACHE["nc"] = _build_nc()
    return _CACHE["nc"]


def _tile_w(w, rows, cols):
    # [R, Cc] -> [R/128, Cc/512, 128, 512] contiguous tiles
    r, c = w.shape
    return np.ascontiguousarray(
        w.reshape(r // 128, 128, c // 512, 512).transpose(0, 2, 1, 3))


def _make_in_maps(inputs):
    import ml_dtypes
    bf = ml_dtypes.bfloat16
    x = np.asarray(inputs["x"], np.float32)
    mask = np.asarray(inputs["padding_mask"]).astype(bool)
    alibi = np.asarray(inputs["alibi_bias"], np.float32)
    wqkv = np.asarray(inputs["Wqkv"], np.float32)
    bqkv = np.asarray(inputs["bqkv"], np.float32)
    wproj = np.asarray(inputs["Wproj"], np.float32)
    bproj = np.asarray(inputs["bproj"], np.float32)
    w1 = np.asarray(inputs["W1"], np.float32)
    b1 = np.asarray(inputs["b1"], np.float32)
    w2 = np.asarray(inputs["W2"], np.float32)
    b2 = np.asarray(inputs["b2"], np.float32)
    ls = np.asarray(inputs["logit_scale"], np.float32).reshape(H)
    scale = np.exp(np.minimum(ls, math.log(100.0))).astype(np.float32)
    amax = float(alibi.max())
    bound = scale + amax + 1.0
    nbound = np.ascontiguousarray(np.tile((-bound).astype(np.float32)[None, :],
                                          (128, 1)))
    consts = np.zeros((128, 384), dtype=np.float32)
    consts[:, 0:128] = np.eye(128, dtype=np.float32)
    consts[:, 128:192] = 1.0
    consts[0:64, 192] = 1.0
    consts[64:128, 193] = 1.0
    consts[0, 194:258] = 1.0
    consts[1, 258:322] = 1.0
    consts = np.ascontiguousarray(consts)
    scales_bc = np.zeros((2, 8, 256), dtype=np.float32)
    for h in range(H):
        scales_bc[h % 2, h // 2, :] = scale[h]

    common = {
        "wqkv_t": _tile_w(wqkv, C, 3 * C).astype(bf),
        "bqkv_t": np.ascontiguousarray(bqkv.reshape(24, 128).T),
        "wproj_t": _tile_w(wproj, C, C).astype(bf),
        "bproj_t": np.ascontiguousarray(bproj.reshape(8, 128).T),
        "w1_t": _tile_w(w1, C, HID).astype(bf),
        "b1_t": np.ascontiguousarray(b1.reshape(32, 128).T),
        "w2_t": _tile_w(w2, HID, C).astype(bf),
        "b2_t": np.ascontiguousarray(b2.reshape(8, 128).T),
        "ln1_g": np.asarray(inputs["ln1_g"], np.float32).reshape(1, C),
        "ln1_b": np.asarray(inputs["ln1_b"], np.float32).reshape(1, C),
        "ln2_g": np.asarray(inputs["ln2_g"], np.float32).reshape(1, C),
        "ln2_b": np.asarray(inputs["ln2_b"], np.float32).reshape(1, C),
        "scales": scales_bc,
        "nbound": nbound,
        "consts": consts,
        "consts_bf": consts.astype(bf),
    }
    in_maps = []
    for c in range(NCORES):
        b, qi = divmod(c, GROUP)
        q0 = qi * TLOC
        alT = alibi[b, :, q0:q0 + TLOC, :].transpose(0, 2, 1)  # [H, N(k), TLOC]
        alT = alT + np.where(mask[b], np.float32(-1e9),
                             np.float32(0.0)).astype(np.float32)[None, :, None]
        alT = np.ascontiguousarray(
            alT.reshape(H, 8, 128, TLOC)).astype(bf)
        m = dict(common)
        m["x_loc"] = np.ascontiguousarray(x[b, q0:q0 + TLOC, :])
        m["alibi_t"] = alT
        in_maps.append(m)
    return in_maps


def _run(inputs, trace=False):
    from concourse import bass_utils
    nc = _get_nc()
    in_maps = _make_in_maps(inputs)
    res = bass_utils.run_bass_kernel_spmd(
        nc, in_maps, core_ids=list(range(NCORES)), trace=trace)
    outs = [np.asarray(res.results[c]["out"]) for c in range(NCORES)]
    y = np.stack(outs).reshape(B, GROUP * TLOC, C)
    return y.astype(np.float32), res


def kernel(**inputs):
    y, _ = _run(inputs, trace=False)
    return y
